# revision 13
# baseline (speedup 1.0000x reference)
"""EvolveGCN-O kernel for Trainium2 (8 NeuronCores).

Key algebraic restructure: the reference keeps, for node i, only the logits
computed at timestep t_i = time_step[i].  The GCN aggregation at time t is
linear in x, so

  logits_i = cls( relu( (sum_{j->i active@t_i} norm_ji x_j + x_i/deg_i) @ W_{t_i} @ proj^T + b ) )

with norm/deg computed from in-degree counts at t_i.  So instead of 49 full
GCN passes we do ONE edge-aggregation pass (over edges (j,i) with
t_j <= t_i) and one per-timestep-group matmul with P_t = W_t @ proj^T.

Sharding (METIS-style partition + halo exchange): nodes are partitioned
across 8 cores by (t, core); each core receives the deduplicated "halo" set
of x rows its edges reference, laid out in first-use order so the edge
aggregation streams it SEQUENTIALLY (no per-row descriptors).  Only repeated
sources (~9% of edges) are fetched by on-device indirect gathers.

Device work per core:
  stage 1: s^T accumulation: psum += slab_chunk^T @ onehot(dst slot, w_e)
           for primary edges; per-group indirect-gathered dup chunks add the
           repeated-source edges; the self term (sw_i * x_i)^T is streamed
           pre-transposed and merged during the PSUM->SBUF copy (DVE add).
  stage 2: z^T = relu(P_t^T s^T + b)   (t static per tile group)
  stage 3: lg^T = cls_w^T^T z^T, stores batched over 7 t-groups
Host does: GRU weight evolution (tiny FxF chain), degree tables, edge
weights, graph partitioning / relabeling / halo tables, unpermute + cls bias.
"""

import ml_dtypes
import numpy as np

N, E, F, H, C, T = 200000, 500000, 166, 128, 2, 49
NCORES = 8
S = 640                      # per-core slots per timestep group (5 tiles)
TILES_PER_T = S // 128       # 5
NT_TILES = T * TILES_PER_T   # 245
NPAD = T * S                 # 31360 slots per core
F1 = 128                     # feature chunk 1
F2 = F - F1                  # 38
GBATCH = 7                   # t-groups per output store

_cache = {}


def _gru_step(Wm, w_ih, w_hh, b_ih, b_hh):
    gi = Wm @ w_ih.T + b_ih
    gh = Wm @ w_hh.T + b_hh
    i_r, i_z, i_n = np.split(gi, 3, axis=-1)
    h_r, h_z, h_n = np.split(gh, 3, axis=-1)
    r = 1.0 / (1.0 + np.exp(-(i_r + h_r)))
    z = 1.0 / (1.0 + np.exp(-(i_z + h_z)))
    nn_ = np.tanh(i_n + r * h_n)
    return (1.0 - z) * nn_ + z * Wm


def _host_prep(x, edge_index, time_step, initial_w, gru_w_ih, gru_w_hh,
               gru_b_ih, gru_b_hh, proj_w, proj_b, cls_w, cls_b):
    src = edge_index[0].astype(np.int64)
    dst = edge_index[1].astype(np.int64)
    t = time_step.astype(np.int64)

    # --- evolve W, fuse with proj ---
    Wm = initial_w.astype(np.float64)
    w_ih = gru_w_ih.astype(np.float64)
    w_hh = gru_w_hh.astype(np.float64)
    b_ih = gru_b_ih.astype(np.float64)
    b_hh = gru_b_hh.astype(np.float64)
    P_stack = np.empty((T, F, H), np.float32)
    projT = proj_w.T.astype(np.float64)
    for step in range(T):
        Wm = _gru_step(Wm, w_ih, w_hh, b_ih, b_hh)
        P_stack[step] = (Wm @ projT).astype(np.float32)

    # --- in-degree table C[v, tau] = #edges (k,v) with t_k <= tau ---
    flat = dst * T + t[src]
    hist = np.bincount(flat, minlength=N * T).astype(np.int32).reshape(N, T)
    Ccum = np.cumsum(hist, axis=1, dtype=np.int32)

    td = t[dst]
    active = t[src] <= td
    deg_dst = Ccum[dst, td] + 1
    deg_src = Ccum[src, td] + 1          # valid where active
    w_e = np.where(active,
                   1.0 / np.sqrt(deg_src.astype(np.float64) * deg_dst.astype(np.float64)),
                   0.0).astype(np.float32)
    sw = (1.0 / (Ccum[np.arange(N), t] + 1.0)).astype(np.float32)  # self weight

    # --- relabel nodes by (t, core, position) ---
    act_indeg = np.bincount(dst[active], minlength=N)
    order = np.argsort(t, kind="stable")          # grouped by t
    counts = np.bincount(t, minlength=T)
    starts = np.concatenate(([0], np.cumsum(counts)))[:-1]
    slot_core = np.empty(N, np.int32)
    slot_idx = np.empty(N, np.int32)
    orig_of = np.full((NCORES, NPAD), -1, np.int64)
    for tt in range(T):
        grp = order[starts[tt]: starts[tt] + counts[tt]]
        # ascending-degree round-robin: every core gets a near-identical
        # degree profile, so per-tile chunk counts align across cores
        grp = grp[np.argsort(act_indeg[grp], kind="stable")]
        for c in range(NCORES):
            seg = grp[c::NCORES]
            k = len(seg)
            assert k <= S, f"t-group {tt} core {c} has {k} > S={S} nodes"
            pos2 = np.arange(k)
            slot_core[seg] = c
            slot_idx[seg] = (tt * S + pos2).astype(np.int32)
            orig_of[c, tt * S + pos2] = seg

    # --- self rows, pre-scaled + transposed: xrT1 [128, NT*128], xrT2 [38, NT*128]
    xf = x.astype(np.float32)
    xrT1_cores, xrT2_cores = [], []
    for c in range(NCORES):
        ids = orig_of[c]
        valid = ids >= 0
        xr = np.zeros((NPAD, F), np.float32)
        xr[valid] = xf[ids[valid]] * sw[ids[valid]][:, None]
        xr3 = xr.reshape(NT_TILES, 128, F)
        xrT1_cores.append(np.ascontiguousarray(
            xr3[:, :, 0:F1].transpose(2, 0, 1).reshape(F1, NT_TILES * 128)
        ).astype(ml_dtypes.bfloat16))
        xrT2_cores.append(np.ascontiguousarray(
            xr3[:, :, F1:F].transpose(2, 0, 1).reshape(F2, NT_TILES * 128)
        ).astype(ml_dtypes.bfloat16))

    # --- per-core edge streams: split primary (first use of src) vs dup ---
    a_idx = np.nonzero(active)[0]
    e_src_a = src[a_idx]
    e_w_a = w_e[a_idx]
    e_core_a = slot_core[dst[a_idx]]
    e_slot_a = slot_idx[dst[a_idx]]

    x_bf = x.astype(ml_dtypes.bfloat16)
    prim = []            # per core: (src, w, slot) arrays for primary edges
    dups = []            # per core: (src, w, slot) arrays for dup edges
    prim_counts = np.zeros((NCORES, NT_TILES), np.int64)
    for c in range(NCORES):
        m = e_core_a == c
        s_c, w_c, sl_c = e_src_a[m], e_w_a[m], e_slot_a[m]
        o = np.argsort(sl_c, kind="stable")
        s_c, w_c, sl_c = s_c[o], w_c[o], sl_c[o]
        _, first_i = np.unique(s_c, return_index=True)
        is_prim = np.zeros(len(s_c), bool)
        is_prim[first_i] = True
        prim.append((s_c[is_prim], w_c[is_prim], sl_c[is_prim]))
        dups.append((s_c[~is_prim], w_c[~is_prim], sl_c[~is_prim]))
        prim_counts[c] = np.bincount(sl_c[is_prim] // 128, minlength=NT_TILES)

    klist = np.ceil(prim_counts.max(axis=0) / 128).astype(np.int64)
    col_base = np.concatenate(([0], np.cumsum(klist)))
    ECH = int(col_base[-1])

    # primary chunk tables + slab + first-use position of each source
    slab_cores, ewT_cores, elidT_cores = [], [], []
    fpos_cores = []
    for c in range(NCORES):
        s_c, w_c, sl_c = prim[c]
        ti_c = sl_c // 128
        rank = np.arange(len(s_c)) - np.concatenate(
            ([0], np.cumsum(np.bincount(ti_c, minlength=NT_TILES))))[:-1][ti_c]
        cidx = col_base[ti_c] + rank // 128
        part = rank % 128
        slab = np.zeros((128, ECH * F), ml_dtypes.bfloat16)
        slab[part[:, None], (cidx * F)[:, None] + np.arange(F)] = x_bf[s_c]
        ewT = np.zeros((128, ECH), np.float32)
        elidT = np.zeros((128, ECH), np.int64)
        ewT[part, cidx] = w_c
        elidT[part, cidx] = sl_c % 128
        slab_cores.append(slab)
        ewT_cores.append(ewT)
        elidT_cores.append(elidT)
        # dup gather row index into slab viewed as [128*ECH, F]: p*ECH + cidx
        fpos = dict(zip(s_c.tolist(), (part.astype(np.int64) * ECH + cidx).tolist()))
        fpos_cores.append(fpos)

    # --- dup chunks: per t-group, sorted by slot, chunked by 128 (SPMD-common) ---
    dup_by_gc = [[None] * NCORES for _ in range(T)]
    dg_counts = np.zeros((NCORES, T), np.int64)
    for c in range(NCORES):
        s_c, w_c, sl_c = dups[c]
        g_c = sl_c // S
        o = np.lexsort((sl_c, g_c))
        s_c, w_c, sl_c, g_c = s_c[o], w_c[o], sl_c[o], g_c[o]
        for g in range(T):
            m = g_c == g
            dup_by_gc[g][c] = (s_c[m], w_c[m], sl_c[m])
            dg_counts[c, g] = m.sum()
    DG = dg_counts.max(axis=0)
    dch_group = []       # group of each dup chunk
    dch_spans = []       # tuple of ti values per dup chunk
    for g in range(T):
        for k in range(int(np.ceil(DG[g] / 128))):
            span = set()
            for c in range(NCORES):
                sl = dup_by_gc[g][c][2][k * 128:(k + 1) * 128]
                span.update((sl // 128).tolist())
            if not span:
                continue
            dch_group.append((g, k))
            dch_spans.append(tuple(sorted(int(v) for v in span)))
    NDCH = len(dch_group)
    NSPAN = sum(len(s) for s in dch_spans)

    dupidx_cores = np.zeros((NCORES, 128, max(NDCH, 1)), np.int32)
    ewd_cores = np.zeros((NCORES, 128, max(NSPAN, 1)), np.float32)
    elidd_cores = np.zeros((NCORES, 128, max(NSPAN, 1)), np.int64)
    spi = 0
    for d in range(NDCH):
        g, k = dch_group[d]
        for c in range(NCORES):
            s_c, w_c, sl_c = dup_by_gc[g][c]
            s_k = s_c[k * 128:(k + 1) * 128]
            w_k = w_c[k * 128:(k + 1) * 128]
            sl_k = sl_c[k * 128:(k + 1) * 128]
            nk = len(s_k)
            fp = fpos_cores[c]
            if nk:
                dupidx_cores[c, :nk, d] = [fp[int(sv)] for sv in s_k]
            for si, ti in enumerate(dch_spans[d]):
                mspan = (sl_k // 128) == ti
                col = spi + si
                ewd_cores[c, :nk, col] = np.where(mspan, w_k, 0.0)
                elidd_cores[c, :nk, col] = np.where(mspan, sl_k % 128, 0)
        spi += len(dch_spans[d])

    # ---- pack dense one-hot stream in device consumption order ----
    ti_spans_h = [[] for _ in range(NT_TILES)]
    spi = 0
    for d in range(NDCH):
        for si, ti in enumerate(dch_spans[d]):
            ti_spans_h[ti].append(spi + si)
        spi += len(dch_spans[d])
    order_cols = []          # (kind, col): kind 0 = primary cidx, 1 = span col
    col_base_h = np.concatenate(([0], np.cumsum(klist)))
    for ti in range(NT_TILES):
        for k in range(int(klist[ti])):
            order_cols.append((0, int(col_base_h[ti]) + k))
        for spcol in ti_spans_h[ti]:
            order_cols.append((1, spcol))
    NOH = len(order_cols)
    oh_cores = []
    rows = np.arange(128)
    for c in range(NCORES):
        ohs = np.zeros((128, NOH * 128), ml_dtypes.bfloat16)
        for q, (kind, col) in enumerate(order_cols):
            if kind == 0:
                wv, lid = ewT_cores[c][:, col], elidT_cores[c][:, col]
            else:
                wv, lid = ewd_cores[c][:, col], elidd_cores[c][:, col]
            ohs[rows, q * 128 + lid] = wv
        oh_cores.append(ohs)

    per_core = []
    for c in range(NCORES):
        per_core.append({
            "slab": slab_cores[c],
            "xrT1": xrT1_cores[c],
            "xrT2": xrT2_cores[c],
            "ohs": oh_cores[c],
            "dupidx": dupidx_cores[c],
            "P_stack": np.ascontiguousarray(
                P_stack.transpose(1, 0, 2).reshape(F, T * H)).astype(ml_dtypes.bfloat16),
            "projb": proj_b.reshape(H, 1).astype(np.float32),
            "clsw": cls_w.T.astype(ml_dtypes.bfloat16).copy(),   # [H, C]
        })
    K = (tuple(int(v) for v in klist), tuple(dch_spans))
    return per_core, orig_of, K


def _build(K):
    import concourse.bacc as bacc
    import concourse.bass as bass
    import concourse.mybir as mybir
    import concourse.tile as tile

    klist, dch_spans = K
    klist = list(klist)
    col_base = [0]
    for v in klist:
        col_base.append(col_base[-1] + v)
    ECH = col_base[-1]
    NDCH = len(dch_spans)
    NSPAN = sum(len(s) for s in dch_spans)
    ti_spans = [[] for _ in range(NT_TILES)]
    spi = 0
    for d in range(NDCH):
        for si, ti in enumerate(dch_spans[d]):
            ti_spans[ti].append((d, spi + si))
        spi += len(dch_spans[d])
    # oh stream slot of each (kind, col), mirroring host packing order
    oh_of = {}
    q = 0
    for ti in range(NT_TILES):
        for k in range(klist[ti]):
            oh_of[(0, col_base[ti] + k)] = q
            q += 1
        for d, spcol in ti_spans[ti]:
            oh_of[(1, spcol)] = q
            q += 1

    nc = bacc.Bacc("TRN2", target_bir_lowering=False, debug=False,
                   num_devices=NCORES)
    dt = mybir.dt.float32
    bf = mybir.dt.bfloat16
    slab_d = nc.dram_tensor("slab", [128, ECH * F], bf, kind="ExternalInput")
    xrT1_d = nc.dram_tensor("xrT1", [F1, NT_TILES * 128], bf, kind="ExternalInput")
    xrT2_d = nc.dram_tensor("xrT2", [F2, NT_TILES * 128], bf, kind="ExternalInput")
    NOH = ECH + NSPAN
    ohs_d = nc.dram_tensor("ohs", [128, NOH * 128], bf, kind="ExternalInput")
    dupidx_d = nc.dram_tensor("dupidx", [128, max(NDCH, 1)], mybir.dt.int32,
                              kind="ExternalInput")
    P_d = nc.dram_tensor("P_stack", [F, T * H], bf, kind="ExternalInput")
    projb_d = nc.dram_tensor("projb", [H, 1], dt, kind="ExternalInput")
    clsw_d = nc.dram_tensor("clsw", [H, C], bf, kind="ExternalInput")
    lgT_d = nc.dram_tensor("lgT", [C, NPAD], dt, kind="ExternalOutput")

    def pieces(total, first, big):
        bounds = [0]
        while bounds[-1] < total:
            step = first if len(bounds) <= 4 else big
            bounds.append(min(total, bounds[-1] + step))
        return bounds

    slab_b = pieces(ECH, 8, 48)
    piece_of_chunk = [0] * ECH
    for pi in range(len(slab_b) - 1):
        for cdx in range(slab_b[pi], slab_b[pi + 1]):
            piece_of_chunk[cdx] = pi
    xrt_b = pieces(NT_TILES, 10, 50)
    piece_of_tile = [0] * NT_TILES
    for pi in range(len(xrt_b) - 1):
        for tix in range(xrt_b[pi], xrt_b[pi + 1]):
            piece_of_tile[tix] = pi

    with tile.TileContext(nc) as tc:
        with (
            tc.tile_pool(name="const", bufs=1) as cpool,
            tc.tile_pool(name="meta", bufs=1) as mpool,
            tc.tile_pool(name="slab", bufs=3) as slabpool,
            tc.tile_pool(name="ohslab", bufs=2) as ohslabpool,
            tc.tile_pool(name="xrt", bufs=2) as xrtpool,
            tc.tile_pool(name="yd", bufs=1) as ydpool,
            tc.tile_pool(name="oh", bufs=12) as ohpool,
            tc.tile_pool(name="st", bufs=2) as stpool,
            tc.tile_pool(name="zt", bufs=2) as ztpool,
            tc.tile_pool(name="lg", bufs=1) as lgpool,
            tc.tile_pool(name="ps", bufs=4, space="PSUM") as pspool,
            tc.tile_pool(name="ps2", bufs=2, space="PSUM") as ps2pool,
            tc.tile_pool(name="pz", bufs=2, space="PSUM") as pzpool,
        ):
            projb_sb = cpool.tile([H, 1], dt)
            nc.sync.dma_start(out=projb_sb[:], in_=projb_d[:])
            clsw_sb = cpool.tile([H, C], bf)
            nc.sync.dma_start(out=clsw_sb[:], in_=clsw_d[:])
            dupidx_sb = mpool.tile([128, max(NDCH, 1)], mybir.dt.int32)
            nc.sync.dma_start(out=dupidx_sb[:], in_=dupidx_d[:])
            # dup-chunk gathers (from the DRAM slab, row view [128*ECH, F])
            slab_rows = slab_d[:].rearrange("p (c f) -> (p c) f", f=F)
            ydup = []
            for d in range(NDCH):
                y = ydpool.tile([128, F], bf, tag=f"yd{d}")
                nc.gpsimd.indirect_dma_start(
                    out=y[:], out_offset=None, in_=slab_rows,
                    in_offset=bass.IndirectOffsetOnAxis(
                        ap=dupidx_sb[:, d:d + 1], axis=0),
                )
                ydup.append(y)

            oh_b = pieces(NOH, 10, 72)
            piece_of_oh = [0] * NOH
            for pi in range(len(oh_b) - 1):
                for qx in range(oh_b[pi], oh_b[pi + 1]):
                    piece_of_oh[qx] = pi
            OHW = max(oh_b[i + 1] - oh_b[i] for i in range(len(oh_b) - 1))
            SLW = max(slab_b[i + 1] - slab_b[i] for i in range(len(slab_b) - 1))
            XRW = max(xrt_b[i + 1] - xrt_b[i] for i in range(len(xrt_b) - 1))

            # interleave initial pieces so tile 0's inputs land first
            ohslabs, slabs, xrt1s, xrt2s = [], [], [], []
            nmax = max(len(oh_b), len(slab_b), len(xrt_b)) - 1
            for pi in range(nmax):
                if pi < len(slab_b) - 1:
                    c0, c1 = slab_b[pi], slab_b[pi + 1]
                    stile = slabpool.tile([128, SLW * F], bf, tag="slab")
                    nc.sync.dma_start(out=stile[:, 0:(c1 - c0) * F],
                                      in_=slab_d[:, c0 * F:c1 * F])
                    slabs.append(stile)
                if pi < len(oh_b) - 1:
                    c0, c1 = oh_b[pi], oh_b[pi + 1]
                    otile = ohslabpool.tile([128, OHW * 128], bf, tag="ohslab")
                    nc.sync.dma_start(out=otile[:, 0:(c1 - c0) * 128],
                                      in_=ohs_d[:, c0 * 128:c1 * 128])
                    ohslabs.append(otile)
                if pi < len(xrt_b) - 1:
                    c0, c1 = xrt_b[pi], xrt_b[pi + 1]
                    x1 = xrtpool.tile([F1, XRW * 128], bf, tag="xrt1")
                    nc.sync.dma_start(out=x1[:, 0:(c1 - c0) * 128],
                                      in_=xrT1_d[:, c0 * 128:c1 * 128])
                    x2 = xrtpool.tile([F2, XRW * 128], bf, tag="xrt2")
                    nc.sync.dma_start(out=x2[:, 0:(c1 - c0) * 128],
                                      in_=xrT2_d[:, c0 * 128:c1 * 128])
                    xrt1s.append(x1)
                    xrt2s.append(x2)

            P1_sb = mpool.tile([F1, T * H], bf)
            P2_sb = mpool.tile([F2, T * H], bf)
            for pp in range(4):
                nc.sync.dma_start(out=P1_sb[:, pp * 1568:(pp + 1) * 1568],
                                  in_=P_d[0:F1, pp * 1568:(pp + 1) * 1568])
                nc.sync.dma_start(out=P2_sb[:, pp * 1568:(pp + 1) * 1568],
                                  in_=P_d[F1:F, pp * 1568:(pp + 1) * 1568])

            for ti in range(NT_TILES):
                tt = ti // TILES_PER_T
                j = ti % TILES_PER_T
                kti = klist[ti]
                spans = ti_spans[ti]
                nmm = kti + len(spans)
                xpi = piece_of_tile[ti]
                x1 = xrt1s[xpi]
                x2 = xrt2s[xpi]
                xo = (ti - xrt_b[xpi]) * 128
                if j == 0:
                    sT1q = stpool.tile([128, S], bf, tag="sT1q")
                    sT2q = stpool.tile([128, S], bf, tag="sT2q")
                if nmm == 0:
                    nc.vector.tensor_copy(out=sT1q[:, j * 128:(j + 1) * 128],
                                          in_=x1[:, xo:xo + 128])
                    nc.scalar.copy(out=sT2q[0:F2, j * 128:(j + 1) * 128],
                                   in_=x2[:, xo:xo + 128])
                else:
                    psum_s = pspool.tile([128, 128], dt, space="PSUM")
                    psum_s2 = ps2pool.tile([F2, 128], dt, space="PSUM")
                    # ops: (lhsT source, oh stream slot) per chunk, F1 pass then F2
                    ops = []
                    for k in range(kti):
                        cidx = col_base[ti] + k
                        spi_ = piece_of_chunk[cidx]
                        ysl = slabs[spi_]
                        off = (cidx - slab_b[spi_]) * F
                        q = oh_of[(0, cidx)]
                        ops.append((ysl, off, q))
                    for d, spcol in spans:
                        ops.append((ydup[d], 0, oh_of[(1, spcol)]))
                    for i, (ysl, off, q) in enumerate(ops):
                        opi = piece_of_oh[q]
                        ohsl = ohslabs[opi]
                        oho = (q - oh_b[opi]) * 128
                        nc.tensor.matmul(out=psum_s[:], lhsT=ysl[:, off:off + F1],
                                         rhs=ohsl[:, oho:oho + 128],
                                         start=i == 0, stop=i == len(ops) - 1)
                    for i, (ysl, off, q) in enumerate(ops):
                        opi = piece_of_oh[q]
                        ohsl = ohslabs[opi]
                        oho = (q - oh_b[opi]) * 128
                        nc.tensor.matmul(out=psum_s2[:], lhsT=ysl[:, off + F1:off + F],
                                         rhs=ohsl[:, oho:oho + 128],
                                         start=i == 0, stop=i == len(ops) - 1)
                    # merge self term during PSUM -> SBUF copy
                    nc.vector.tensor_tensor(
                        out=sT1q[:, j * 128:(j + 1) * 128], in0=psum_s[:],
                        in1=x1[:, xo:xo + 128], op=mybir.AluOpType.add)
                    nc.vector.tensor_tensor(
                        out=sT2q[0:F2, j * 128:(j + 1) * 128], in0=psum_s2[:],
                        in1=x2[:, xo:xo + 128], op=mybir.AluOpType.add)
                if j == TILES_PER_T - 1:
                    # ---- stage 2 batched: z^T = relu(P_t^T s^T + b)
                    pz_a = pzpool.tile([128, 512], dt, space="PSUM", tag="pz")
                    pz_b = pzpool.tile([128, S - 512], dt, space="PSUM", tag="pz")
                    pc0 = tt * H
                    nc.tensor.matmul(out=pz_a[:], lhsT=P1_sb[:, pc0:pc0 + H],
                                     rhs=sT1q[:, 0:512], start=True, stop=False)
                    nc.tensor.matmul(out=pz_b[:], lhsT=P1_sb[:, pc0:pc0 + H],
                                     rhs=sT1q[:, 512:S], start=True, stop=False)
                    nc.tensor.matmul(out=pz_a[:], lhsT=P2_sb[:, pc0:pc0 + H],
                                     rhs=sT2q[0:F2, 0:512], start=False, stop=True)
                    nc.tensor.matmul(out=pz_b[:], lhsT=P2_sb[:, pc0:pc0 + H],
                                     rhs=sT2q[0:F2, 512:S], start=False, stop=True)
                    zTq = ztpool.tile([128, S], bf, tag="zTq")
                    nc.scalar.activation(out=zTq[:, 0:512], in_=pz_a[:],
                                         func=mybir.ActivationFunctionType.Relu,
                                         bias=projb_sb[:, 0:1])
                    nc.scalar.activation(out=zTq[:, 512:S], in_=pz_b[:],
                                         func=mybir.ActivationFunctionType.Relu,
                                         bias=projb_sb[:, 0:1])
                    # ---- stage 3 batched: lg^T for the whole group
                    if tt % GBATCH == 0:
                        lgb = lgpool.tile([C, GBATCH * S], dt, tag="lgb")
                    lo = (tt % GBATCH) * S
                    psum_lg = pzpool.tile([C, 512], dt, space="PSUM", tag="pz")
                    nc.tensor.matmul(out=psum_lg[:], lhsT=clsw_sb[:],
                                     rhs=zTq[:, 0:512], start=True, stop=True)
                    nc.scalar.copy(out=lgb[:, lo:lo + 512], in_=psum_lg[:])
                    psum_lg2 = pzpool.tile([C, 512], dt, space="PSUM", tag="pz")
                    nc.tensor.matmul(out=psum_lg2[:, 0:S - 512], lhsT=clsw_sb[:],
                                     rhs=zTq[:, 512:S], start=True, stop=True)
                    nc.scalar.copy(out=lgb[:, lo + 512:lo + S],
                                   in_=psum_lg2[:, 0:S - 512])
                    if tt % GBATCH == GBATCH - 1 or tt == T - 1:
                        b0 = (tt - tt % GBATCH) * S
                        nc.sync.dma_start(
                            out=lgT_d[:, b0:b0 + (tt % GBATCH) * S + S],
                            in_=lgb[:, 0:(tt % GBATCH) * S + S])
    nc.compile()
    return nc


def kernel(**inputs):
    from concourse.bass_utils import run_bass_kernel_spmd

    np_inputs = {k: np.asarray(v) for k, v in inputs.items()}
    per_core, orig_of, K = _host_prep(**np_inputs)

    if K not in _cache:
        _cache[K] = _build(K)
    nc = _cache[K]

    res = run_bass_kernel_spmd(nc, per_core, list(range(NCORES)))

    cls_b = np_inputs["cls_b"].astype(np.float32)
    logits = np.zeros((N, C), np.float32)
    for c in range(NCORES):
        ids = orig_of[c]
        valid = ids >= 0
        lgT = res.results[c]["lgT"]                    # [C, NPAD]
        logits[ids[valid]] = lgT.T[valid]
    logits += cls_b
    return logits


# revision 14
# speedup vs baseline: 1.1045x; 1.1045x over previous
"""EvolveGCN-O kernel for Trainium2 (8 NeuronCores).

Key algebraic restructure: the reference keeps, for node i, only the logits
computed at timestep t_i = time_step[i].  The GCN aggregation at time t is
linear in x, so

  logits_i = cls( relu( (sum_{j->i active@t_i} norm_ji x_j + x_i/deg_i) @ W_{t_i} @ proj^T + b ) )

with norm/deg computed from in-degree counts at t_i.  So instead of 49 full
GCN passes we do ONE edge-aggregation pass (over edges (j,i) with
t_j <= t_i) and one per-timestep-group matmul with P_t = W_t @ proj^T.

Sharding (METIS-style partition + halo exchange): nodes are partitioned
across 8 cores by (t, core); each core receives the deduplicated "halo" set
of x rows its edges reference, laid out in first-use order so the edge
aggregation streams it SEQUENTIALLY (no per-row descriptors).  Only repeated
sources (~9% of edges) are fetched by on-device indirect gathers.

Device work per core:
  stage 1: s^T accumulation: psum += slab_chunk^T @ onehot(dst slot, w_e)
           for primary edges; per-group indirect-gathered dup chunks add the
           repeated-source edges; the self term (sw_i * x_i)^T is streamed
           pre-transposed and merged during the PSUM->SBUF copy (DVE add).
  stage 2: z^T = relu(P_t^T s^T + b)   (t static per tile group)
  stage 3: lg^T = cls_w^T^T z^T, stores batched over 7 t-groups
Host does: GRU weight evolution (tiny FxF chain), degree tables, edge
weights, graph partitioning / relabeling / halo tables, unpermute + cls bias.
"""

import ml_dtypes
import numpy as np

N, E, F, H, C, T = 200000, 500000, 166, 128, 2, 49
NCORES = 8
S = 640                      # per-core slots per timestep group (5 tiles)
TILES_PER_T = S // 128       # 5
NT_TILES = T * TILES_PER_T   # 245
NPAD = T * S                 # 31360 slots per core
F1 = 128                     # feature chunk 1
F2 = F - F1                  # 38
GBATCH = 7                   # t-groups per output store

_cache = {}


def _gru_step(Wm, w_ih, w_hh, b_ih, b_hh):
    gi = Wm @ w_ih.T + b_ih
    gh = Wm @ w_hh.T + b_hh
    i_r, i_z, i_n = np.split(gi, 3, axis=-1)
    h_r, h_z, h_n = np.split(gh, 3, axis=-1)
    r = 1.0 / (1.0 + np.exp(-(i_r + h_r)))
    z = 1.0 / (1.0 + np.exp(-(i_z + h_z)))
    nn_ = np.tanh(i_n + r * h_n)
    return (1.0 - z) * nn_ + z * Wm


def _host_prep(x, edge_index, time_step, initial_w, gru_w_ih, gru_w_hh,
               gru_b_ih, gru_b_hh, proj_w, proj_b, cls_w, cls_b):
    src = edge_index[0].astype(np.int64)
    dst = edge_index[1].astype(np.int64)
    t = time_step.astype(np.int64)

    # --- evolve W, fuse with proj ---
    Wm = initial_w.astype(np.float64)
    w_ih = gru_w_ih.astype(np.float64)
    w_hh = gru_w_hh.astype(np.float64)
    b_ih = gru_b_ih.astype(np.float64)
    b_hh = gru_b_hh.astype(np.float64)
    P_stack = np.empty((T, F, H), np.float32)
    projT = proj_w.T.astype(np.float64)
    for step in range(T):
        Wm = _gru_step(Wm, w_ih, w_hh, b_ih, b_hh)
        P_stack[step] = (Wm @ projT).astype(np.float32)

    # --- in-degree table C[v, tau] = #edges (k,v) with t_k <= tau ---
    flat = dst * T + t[src]
    hist = np.bincount(flat, minlength=N * T).astype(np.int32).reshape(N, T)
    Ccum = np.cumsum(hist, axis=1, dtype=np.int32)

    td = t[dst]
    active = t[src] <= td
    deg_dst = Ccum[dst, td] + 1
    deg_src = Ccum[src, td] + 1          # valid where active
    w_e = np.where(active,
                   1.0 / np.sqrt(deg_src.astype(np.float64) * deg_dst.astype(np.float64)),
                   0.0).astype(np.float32)
    sw = (1.0 / (Ccum[np.arange(N), t] + 1.0)).astype(np.float32)  # self weight

    # --- relabel nodes by (t, core, position) ---
    act_indeg = np.bincount(dst[active], minlength=N)
    order = np.argsort(t, kind="stable")          # grouped by t
    counts = np.bincount(t, minlength=T)
    starts = np.concatenate(([0], np.cumsum(counts)))[:-1]
    slot_core = np.empty(N, np.int32)
    slot_idx = np.empty(N, np.int32)
    orig_of = np.full((NCORES, NPAD), -1, np.int64)
    for tt in range(T):
        grp = order[starts[tt]: starts[tt] + counts[tt]]
        # ascending-degree round-robin: every core gets a near-identical
        # degree profile, so per-tile chunk counts align across cores
        grp = grp[np.argsort(act_indeg[grp], kind="stable")]
        for c in range(NCORES):
            seg = grp[c::NCORES]
            k = len(seg)
            assert k <= S, f"t-group {tt} core {c} has {k} > S={S} nodes"
            pos2 = np.arange(k)
            slot_core[seg] = c
            slot_idx[seg] = (tt * S + pos2).astype(np.int32)
            orig_of[c, tt * S + pos2] = seg

    # --- self rows, pre-scaled + transposed: xrT1 [128, NT*128], xrT2 [38, NT*128]
    xf = x.astype(np.float32)
    xrT1_cores, xrT2_cores = [], []
    for c in range(NCORES):
        ids = orig_of[c]
        valid = ids >= 0
        xr = np.zeros((NPAD, F), np.float32)
        xr[valid] = xf[ids[valid]] * sw[ids[valid]][:, None]
        xr3 = xr.reshape(NT_TILES, 128, F)
        xrT1_cores.append(np.ascontiguousarray(
            xr3[:, :, 0:F1].transpose(2, 0, 1).reshape(F1, NT_TILES * 128)
        ).astype(ml_dtypes.bfloat16))
        xrT2_cores.append(np.ascontiguousarray(
            xr3[:, :, F1:F].transpose(2, 0, 1).reshape(F2, NT_TILES * 128)
        ).astype(ml_dtypes.bfloat16))

    # --- per-core edge streams: split primary (first use of src) vs dup ---
    a_idx = np.nonzero(active)[0]
    e_src_a = src[a_idx]
    e_w_a = w_e[a_idx]
    e_core_a = slot_core[dst[a_idx]]
    e_slot_a = slot_idx[dst[a_idx]]

    x_bf = x.astype(ml_dtypes.bfloat16)
    prim = []            # per core: (src, w, slot) arrays for primary edges
    dups = []            # per core: (src, w, slot) arrays for dup edges
    prim_counts = np.zeros((NCORES, NT_TILES), np.int64)
    for c in range(NCORES):
        m = e_core_a == c
        s_c, w_c, sl_c = e_src_a[m], e_w_a[m], e_slot_a[m]
        o = np.argsort(sl_c, kind="stable")
        s_c, w_c, sl_c = s_c[o], w_c[o], sl_c[o]
        _, first_i = np.unique(s_c, return_index=True)
        is_prim = np.zeros(len(s_c), bool)
        is_prim[first_i] = True
        prim.append((s_c[is_prim], w_c[is_prim], sl_c[is_prim]))
        dups.append((s_c[~is_prim], w_c[~is_prim], sl_c[~is_prim]))
        prim_counts[c] = np.bincount(sl_c[is_prim] // 128, minlength=NT_TILES)

    klist = np.ceil(prim_counts.max(axis=0) / 128).astype(np.int64)
    col_base = np.concatenate(([0], np.cumsum(klist)))
    ECH = int(col_base[-1])

    # primary chunk tables + slab + first-use position of each source
    slab_cores, ewT_cores, elidT_cores = [], [], []
    fpos_cores = []
    for c in range(NCORES):
        s_c, w_c, sl_c = prim[c]
        ti_c = sl_c // 128
        rank = np.arange(len(s_c)) - np.concatenate(
            ([0], np.cumsum(np.bincount(ti_c, minlength=NT_TILES))))[:-1][ti_c]
        cidx = col_base[ti_c] + rank // 128
        part = rank % 128
        slab = np.zeros((128, ECH * F), ml_dtypes.bfloat16)
        slab[part[:, None], (cidx * F)[:, None] + np.arange(F)] = x_bf[s_c]
        ewT = np.zeros((128, ECH), np.float32)
        elidT = np.zeros((128, ECH), np.int64)
        ewT[part, cidx] = w_c
        elidT[part, cidx] = sl_c % 128
        slab_cores.append(slab)
        ewT_cores.append(ewT)
        elidT_cores.append(elidT)
        # dup gather row index into slab viewed as [128*ECH, F]: p*ECH + cidx
        fpos = dict(zip(s_c.tolist(), (part.astype(np.int64) * ECH + cidx).tolist()))
        fpos_cores.append(fpos)

    # --- dup chunks: per t-group, sorted by slot, chunked by 128 (SPMD-common) ---
    dup_by_gc = [[None] * NCORES for _ in range(T)]
    dg_counts = np.zeros((NCORES, T), np.int64)
    for c in range(NCORES):
        s_c, w_c, sl_c = dups[c]
        g_c = sl_c // S
        o = np.lexsort((sl_c, g_c))
        s_c, w_c, sl_c, g_c = s_c[o], w_c[o], sl_c[o], g_c[o]
        for g in range(T):
            m = g_c == g
            dup_by_gc[g][c] = (s_c[m], w_c[m], sl_c[m])
            dg_counts[c, g] = m.sum()
    DG = dg_counts.max(axis=0)
    dch_group = []       # group of each dup chunk
    dch_spans = []       # tuple of ti values per dup chunk
    for g in range(T):
        for k in range(int(np.ceil(DG[g] / 128))):
            span = set()
            for c in range(NCORES):
                sl = dup_by_gc[g][c][2][k * 128:(k + 1) * 128]
                span.update((sl // 128).tolist())
            if not span:
                continue
            dch_group.append((g, k))
            dch_spans.append(tuple(sorted(int(v) for v in span)))
    NDCH = len(dch_group)
    NSPAN = sum(len(s) for s in dch_spans)

    dupidx_cores = np.zeros((NCORES, 128, max(NDCH, 1)), np.int32)
    ewd_cores = np.zeros((NCORES, 128, max(NSPAN, 1)), np.float32)
    elidd_cores = np.zeros((NCORES, 128, max(NSPAN, 1)), np.int64)
    spi = 0
    for d in range(NDCH):
        g, k = dch_group[d]
        for c in range(NCORES):
            s_c, w_c, sl_c = dup_by_gc[g][c]
            s_k = s_c[k * 128:(k + 1) * 128]
            w_k = w_c[k * 128:(k + 1) * 128]
            sl_k = sl_c[k * 128:(k + 1) * 128]
            nk = len(s_k)
            fp = fpos_cores[c]
            if nk:
                dupidx_cores[c, :nk, d] = [fp[int(sv)] for sv in s_k]
            for si, ti in enumerate(dch_spans[d]):
                mspan = (sl_k // 128) == ti
                col = spi + si
                ewd_cores[c, :nk, col] = np.where(mspan, w_k, 0.0)
                elidd_cores[c, :nk, col] = np.where(mspan, sl_k % 128, 0)
        spi += len(dch_spans[d])

    # ---- pack dense one-hot stream in device consumption order ----
    ti_spans_h = [[] for _ in range(NT_TILES)]
    spi = 0
    for d in range(NDCH):
        for si, ti in enumerate(dch_spans[d]):
            ti_spans_h[ti].append(spi + si)
        spi += len(dch_spans[d])
    order_cols = []          # (kind, col): kind 0 = primary cidx, 1 = span col
    col_base_h = np.concatenate(([0], np.cumsum(klist)))
    for ti in range(NT_TILES):
        for k in range(int(klist[ti])):
            order_cols.append((0, int(col_base_h[ti]) + k))
        for spcol in ti_spans_h[ti]:
            order_cols.append((1, spcol))
    NOH = len(order_cols)
    oh_cores = []
    rows = np.arange(128)
    for c in range(NCORES):
        ohs = np.zeros((128, NOH * 128), ml_dtypes.bfloat16)
        for q, (kind, col) in enumerate(order_cols):
            if kind == 0:
                wv, lid = ewT_cores[c][:, col], elidT_cores[c][:, col]
            else:
                wv, lid = ewd_cores[c][:, col], elidd_cores[c][:, col]
            ohs[rows, q * 128 + lid] = wv
        oh_cores.append(ohs)

    per_core = []
    for c in range(NCORES):
        per_core.append({
            "slab": slab_cores[c],
            "xrT1": xrT1_cores[c],
            "xrT2": xrT2_cores[c],
            "ohs": oh_cores[c],
            "dupidx": dupidx_cores[c],
            "P_stack": np.ascontiguousarray(
                P_stack.transpose(1, 0, 2).reshape(F, T * H)).astype(ml_dtypes.bfloat16),
            "projb": proj_b.reshape(H, 1).astype(np.float32),
            "clsw": cls_w.T.astype(ml_dtypes.bfloat16).copy(),   # [H, C]
        })
    K = (tuple(int(v) for v in klist), tuple(dch_spans))
    return per_core, orig_of, K


def _build(K):
    import concourse.bacc as bacc
    import concourse.bass as bass
    import concourse.mybir as mybir
    import concourse.tile as tile

    klist, dch_spans = K
    klist = list(klist)
    col_base = [0]
    for v in klist:
        col_base.append(col_base[-1] + v)
    ECH = col_base[-1]
    NDCH = len(dch_spans)
    NSPAN = sum(len(s) for s in dch_spans)
    ti_spans = [[] for _ in range(NT_TILES)]
    spi = 0
    for d in range(NDCH):
        for si, ti in enumerate(dch_spans[d]):
            ti_spans[ti].append((d, spi + si))
        spi += len(dch_spans[d])
    # oh stream slot of each (kind, col), mirroring host packing order
    oh_of = {}
    q = 0
    for ti in range(NT_TILES):
        for k in range(klist[ti]):
            oh_of[(0, col_base[ti] + k)] = q
            q += 1
        for d, spcol in ti_spans[ti]:
            oh_of[(1, spcol)] = q
            q += 1

    nc = bacc.Bacc("TRN2", target_bir_lowering=False, debug=False,
                   num_devices=NCORES)
    dt = mybir.dt.float32
    bf = mybir.dt.bfloat16
    slab_d = nc.dram_tensor("slab", [128, ECH * F], bf, kind="ExternalInput")
    xrT1_d = nc.dram_tensor("xrT1", [F1, NT_TILES * 128], bf, kind="ExternalInput")
    xrT2_d = nc.dram_tensor("xrT2", [F2, NT_TILES * 128], bf, kind="ExternalInput")
    NOH = ECH + NSPAN
    ohs_d = nc.dram_tensor("ohs", [128, NOH * 128], bf, kind="ExternalInput")
    dupidx_d = nc.dram_tensor("dupidx", [128, max(NDCH, 1)], mybir.dt.int32,
                              kind="ExternalInput")
    P_d = nc.dram_tensor("P_stack", [F, T * H], bf, kind="ExternalInput")
    projb_d = nc.dram_tensor("projb", [H, 1], dt, kind="ExternalInput")
    clsw_d = nc.dram_tensor("clsw", [H, C], bf, kind="ExternalInput")
    lgT_d = nc.dram_tensor("lgT", [C, NPAD], dt, kind="ExternalOutput")

    def pieces(total, first, big):
        bounds = [0]
        while bounds[-1] < total:
            step = first if len(bounds) <= 4 else big
            bounds.append(min(total, bounds[-1] + step))
        return bounds

    slab_b = pieces(ECH, 8, 48)
    piece_of_chunk = [0] * ECH
    for pi in range(len(slab_b) - 1):
        for cdx in range(slab_b[pi], slab_b[pi + 1]):
            piece_of_chunk[cdx] = pi
    xrt_b = pieces(NT_TILES, 10, 50)
    piece_of_tile = [0] * NT_TILES
    for pi in range(len(xrt_b) - 1):
        for tix in range(xrt_b[pi], xrt_b[pi + 1]):
            piece_of_tile[tix] = pi

    with tile.TileContext(nc) as tc:
        with (
            tc.tile_pool(name="const", bufs=1) as cpool,
            tc.tile_pool(name="meta", bufs=1) as mpool,
            tc.tile_pool(name="slab", bufs=3) as slabpool,
            tc.tile_pool(name="ohslab", bufs=2) as ohslabpool,
            tc.tile_pool(name="xrt", bufs=2) as xrtpool,
            tc.tile_pool(name="yd", bufs=1) as ydpool,
            tc.tile_pool(name="oh", bufs=12) as ohpool,
            tc.tile_pool(name="st", bufs=3) as stpool,
            tc.tile_pool(name="zt", bufs=2) as ztpool,
            tc.tile_pool(name="lg", bufs=1) as lgpool,
            tc.tile_pool(name="ps", bufs=3, space="PSUM") as pspool,
            tc.tile_pool(name="ps2", bufs=3, space="PSUM") as ps2pool,
            tc.tile_pool(name="pz", bufs=2, space="PSUM") as pzpool,
        ):
            projb_sb = cpool.tile([H, 1], dt)
            nc.sync.dma_start(out=projb_sb[:], in_=projb_d[:])
            clsw_sb = cpool.tile([H, C], bf)
            nc.sync.dma_start(out=clsw_sb[:], in_=clsw_d[:])
            dupidx_sb = mpool.tile([128, max(NDCH, 1)], mybir.dt.int32)
            nc.sync.dma_start(out=dupidx_sb[:], in_=dupidx_d[:])
            # dup-chunk gathers (from the DRAM slab, row view [128*ECH, F])
            slab_rows = slab_d[:].rearrange("p (c f) -> (p c) f", f=F)
            ydup = []
            for d in range(NDCH):
                y = ydpool.tile([128, F], bf, tag=f"yd{d}")
                nc.gpsimd.indirect_dma_start(
                    out=y[:], out_offset=None, in_=slab_rows,
                    in_offset=bass.IndirectOffsetOnAxis(
                        ap=dupidx_sb[:, d:d + 1], axis=0),
                )
                ydup.append(y)

            oh_b = pieces(NOH, 10, 72)
            piece_of_oh = [0] * NOH
            for pi in range(len(oh_b) - 1):
                for qx in range(oh_b[pi], oh_b[pi + 1]):
                    piece_of_oh[qx] = pi
            OHW = max(oh_b[i + 1] - oh_b[i] for i in range(len(oh_b) - 1))
            SLW = max(slab_b[i + 1] - slab_b[i] for i in range(len(slab_b) - 1))
            XRW = max(xrt_b[i + 1] - xrt_b[i] for i in range(len(xrt_b) - 1))

            # interleave initial pieces so tile 0's inputs land first
            ohslabs, slabs, xrt1s, xrt2s = [], [], [], []
            nmax = max(len(oh_b), len(slab_b), len(xrt_b)) - 1
            for pi in range(nmax):
                if pi < len(slab_b) - 1:
                    c0, c1 = slab_b[pi], slab_b[pi + 1]
                    stile = slabpool.tile([128, SLW * F], bf, tag="slab")
                    nc.sync.dma_start(out=stile[:, 0:(c1 - c0) * F],
                                      in_=slab_d[:, c0 * F:c1 * F])
                    slabs.append(stile)
                if pi < len(oh_b) - 1:
                    c0, c1 = oh_b[pi], oh_b[pi + 1]
                    otile = ohslabpool.tile([128, OHW * 128], bf, tag="ohslab")
                    nc.sync.dma_start(out=otile[:, 0:(c1 - c0) * 128],
                                      in_=ohs_d[:, c0 * 128:c1 * 128])
                    ohslabs.append(otile)
                if pi < len(xrt_b) - 1:
                    c0, c1 = xrt_b[pi], xrt_b[pi + 1]
                    x1 = xrtpool.tile([F1, XRW * 128], bf, tag="xrt1")
                    nc.sync.dma_start(out=x1[:, 0:(c1 - c0) * 128],
                                      in_=xrT1_d[:, c0 * 128:c1 * 128])
                    x2 = xrtpool.tile([F2, XRW * 128], bf, tag="xrt2")
                    nc.sync.dma_start(out=x2[:, 0:(c1 - c0) * 128],
                                      in_=xrT2_d[:, c0 * 128:c1 * 128])
                    xrt1s.append(x1)
                    xrt2s.append(x2)

            P1_sb = mpool.tile([F1, T * H], bf)
            P2_sb = mpool.tile([F2, T * H], bf)
            for pp in range(4):
                nc.sync.dma_start(out=P1_sb[:, pp * 1568:(pp + 1) * 1568],
                                  in_=P_d[0:F1, pp * 1568:(pp + 1) * 1568])
                nc.sync.dma_start(out=P2_sb[:, pp * 1568:(pp + 1) * 1568],
                                  in_=P_d[F1:F, pp * 1568:(pp + 1) * 1568])

            for ti in range(NT_TILES):
                tt = ti // TILES_PER_T
                j = ti % TILES_PER_T
                kti = klist[ti]
                spans = ti_spans[ti]
                nmm = kti + len(spans)
                xpi = piece_of_tile[ti]
                x1 = xrt1s[xpi]
                x2 = xrt2s[xpi]
                xo = (ti - xrt_b[xpi]) * 128
                if j == 0:
                    sT1q = stpool.tile([128, S], bf, tag="sT1q")
                    sT2q = stpool.tile([128, S], bf, tag="sT2q")
                if nmm == 0:
                    nc.vector.tensor_copy(out=sT1q[:, j * 128:(j + 1) * 128],
                                          in_=x1[:, xo:xo + 128])
                    nc.scalar.copy(out=sT2q[0:F2, j * 128:(j + 1) * 128],
                                   in_=x2[:, xo:xo + 128])
                else:
                    psum_s = pspool.tile([128, 128], dt, space="PSUM")
                    psum_s2 = ps2pool.tile([F2, 128], dt, space="PSUM")
                    # ops: (lhsT source, oh stream slot) per chunk, F1 pass then F2
                    ops = []
                    for k in range(kti):
                        cidx = col_base[ti] + k
                        spi_ = piece_of_chunk[cidx]
                        ysl = slabs[spi_]
                        off = (cidx - slab_b[spi_]) * F
                        q = oh_of[(0, cidx)]
                        ops.append((ysl, off, q))
                    for d, spcol in spans:
                        ops.append((ydup[d], 0, oh_of[(1, spcol)]))
                    for i, (ysl, off, q) in enumerate(ops):
                        opi = piece_of_oh[q]
                        ohsl = ohslabs[opi]
                        oho = (q - oh_b[opi]) * 128
                        nc.tensor.matmul(out=psum_s[:], lhsT=ysl[:, off:off + F1],
                                         rhs=ohsl[:, oho:oho + 128],
                                         start=i == 0, stop=i == len(ops) - 1)
                    for i, (ysl, off, q) in enumerate(ops):
                        opi = piece_of_oh[q]
                        ohsl = ohslabs[opi]
                        oho = (q - oh_b[opi]) * 128
                        nc.tensor.matmul(out=psum_s2[:], lhsT=ysl[:, off + F1:off + F],
                                         rhs=ohsl[:, oho:oho + 128],
                                         start=i == 0, stop=i == len(ops) - 1)
                    # merge self term during PSUM -> SBUF copy
                    nc.vector.tensor_tensor(
                        out=sT1q[:, j * 128:(j + 1) * 128], in0=psum_s[:],
                        in1=x1[:, xo:xo + 128], op=mybir.AluOpType.add)
                    nc.vector.tensor_tensor(
                        out=sT2q[0:F2, j * 128:(j + 1) * 128], in0=psum_s2[:],
                        in1=x2[:, xo:xo + 128], op=mybir.AluOpType.add)
                if j == TILES_PER_T - 1:
                    # ---- stage 2 batched: z^T = relu(P_t^T s^T + b)
                    pz_a = pzpool.tile([128, 512], dt, space="PSUM", tag="pz")
                    pz_b = pzpool.tile([128, S - 512], dt, space="PSUM", tag="pz")
                    pc0 = tt * H
                    nc.tensor.matmul(out=pz_a[:], lhsT=P1_sb[:, pc0:pc0 + H],
                                     rhs=sT1q[:, 0:512], start=True, stop=False)
                    nc.tensor.matmul(out=pz_b[:], lhsT=P1_sb[:, pc0:pc0 + H],
                                     rhs=sT1q[:, 512:S], start=True, stop=False)
                    nc.tensor.matmul(out=pz_a[:], lhsT=P2_sb[:, pc0:pc0 + H],
                                     rhs=sT2q[0:F2, 0:512], start=False, stop=True)
                    nc.tensor.matmul(out=pz_b[:], lhsT=P2_sb[:, pc0:pc0 + H],
                                     rhs=sT2q[0:F2, 512:S], start=False, stop=True)
                    zTq = ztpool.tile([128, S], bf, tag="zTq")
                    nc.scalar.activation(out=zTq[:, 0:512], in_=pz_a[:],
                                         func=mybir.ActivationFunctionType.Relu,
                                         bias=projb_sb[:, 0:1])
                    nc.scalar.activation(out=zTq[:, 512:S], in_=pz_b[:],
                                         func=mybir.ActivationFunctionType.Relu,
                                         bias=projb_sb[:, 0:1])
                    # ---- stage 3 batched: lg^T for the whole group
                    if tt % GBATCH == 0:
                        lgb = lgpool.tile([C, GBATCH * S], dt, tag="lgb")
                    lo = (tt % GBATCH) * S
                    psum_lg = pzpool.tile([C, 512], dt, space="PSUM", tag="pz")
                    nc.tensor.matmul(out=psum_lg[:], lhsT=clsw_sb[:],
                                     rhs=zTq[:, 0:512], start=True, stop=True)
                    nc.scalar.copy(out=lgb[:, lo:lo + 512], in_=psum_lg[:])
                    psum_lg2 = pzpool.tile([C, 512], dt, space="PSUM", tag="pz")
                    nc.tensor.matmul(out=psum_lg2[:, 0:S - 512], lhsT=clsw_sb[:],
                                     rhs=zTq[:, 512:S], start=True, stop=True)
                    nc.scalar.copy(out=lgb[:, lo + 512:lo + S],
                                   in_=psum_lg2[:, 0:S - 512])
                    if tt % GBATCH == GBATCH - 1 or tt == T - 1:
                        b0 = (tt - tt % GBATCH) * S
                        nc.sync.dma_start(
                            out=lgT_d[:, b0:b0 + (tt % GBATCH) * S + S],
                            in_=lgb[:, 0:(tt % GBATCH) * S + S])
    nc.compile()
    return nc


def kernel(**inputs):
    from concourse.bass_utils import run_bass_kernel_spmd

    np_inputs = {k: np.asarray(v) for k, v in inputs.items()}
    per_core, orig_of, K = _host_prep(**np_inputs)

    if K not in _cache:
        _cache[K] = _build(K)
    nc = _cache[K]

    res = run_bass_kernel_spmd(nc, per_core, list(range(NCORES)))

    cls_b = np_inputs["cls_b"].astype(np.float32)
    logits = np.zeros((N, C), np.float32)
    for c in range(NCORES):
        ids = orig_of[c]
        valid = ids >= 0
        lgT = res.results[c]["lgT"]                    # [C, NPAD]
        logits[ids[valid]] = lgT.T[valid]
    logits += cls_b
    return logits


# revision 17
# speedup vs baseline: 1.1276x; 1.0209x over previous
"""EvolveGCN-O kernel for Trainium2 (8 NeuronCores).

Key algebraic restructure: the reference keeps, for node i, only the logits
computed at timestep t_i = time_step[i].  The GCN aggregation at time t is
linear in x, so

  logits_i = cls( relu( (sum_{j->i active@t_i} norm_ji x_j + x_i/deg_i) @ W_{t_i} @ proj^T + b ) )

with norm/deg computed from in-degree counts at t_i.  So instead of 49 full
GCN passes we do ONE edge-aggregation pass (over edges (j,i) with
t_j <= t_i) and one per-timestep-group matmul with P_t = W_t @ proj^T.

Sharding (METIS-style partition + halo exchange): nodes are partitioned
across 8 cores by (t, core); each core receives the deduplicated "halo" set
of x rows its edges reference, laid out in first-use order so the edge
aggregation streams it SEQUENTIALLY (no per-row descriptors).  Only repeated
sources (~9% of edges) are fetched by on-device indirect gathers.

Device work per core:
  stage 1: s^T accumulation: psum += slab_chunk^T @ onehot(dst slot, w_e)
           for primary edges; per-group indirect-gathered dup chunks add the
           repeated-source edges; the self term (sw_i * x_i)^T is streamed
           pre-transposed and merged during the PSUM->SBUF copy (DVE add).
  stage 2: z^T = relu(P_t^T s^T + b)   (t static per tile group)
  stage 3: lg^T = cls_w^T^T z^T, stores batched over 7 t-groups
Host does: GRU weight evolution (tiny FxF chain), degree tables, edge
weights, graph partitioning / relabeling / halo tables, unpermute + cls bias.
"""

import ml_dtypes
import numpy as np

N, E, F, H, C, T = 200000, 500000, 166, 128, 2, 49
NCORES = 8
S = 640                      # per-core slots per timestep group (5 tiles)
TILES_PER_T = S // 128       # 5
NT_TILES = T * TILES_PER_T   # 245
NPAD = T * S                 # 31360 slots per core
F1 = 128                     # feature chunk 1
F2 = F - F1                  # 38
GBATCH = 7                   # t-groups per output store

_cache = {}


def _gru_step(Wm, w_ih, w_hh, b_ih, b_hh):
    gi = Wm @ w_ih.T + b_ih
    gh = Wm @ w_hh.T + b_hh
    i_r, i_z, i_n = np.split(gi, 3, axis=-1)
    h_r, h_z, h_n = np.split(gh, 3, axis=-1)
    r = 1.0 / (1.0 + np.exp(-(i_r + h_r)))
    z = 1.0 / (1.0 + np.exp(-(i_z + h_z)))
    nn_ = np.tanh(i_n + r * h_n)
    return (1.0 - z) * nn_ + z * Wm


def _host_prep(x, edge_index, time_step, initial_w, gru_w_ih, gru_w_hh,
               gru_b_ih, gru_b_hh, proj_w, proj_b, cls_w, cls_b):
    src = edge_index[0].astype(np.int64)
    dst = edge_index[1].astype(np.int64)
    t = time_step.astype(np.int64)

    # --- evolve W, fuse with proj ---
    Wm = initial_w.astype(np.float64)
    w_ih = gru_w_ih.astype(np.float64)
    w_hh = gru_w_hh.astype(np.float64)
    b_ih = gru_b_ih.astype(np.float64)
    b_hh = gru_b_hh.astype(np.float64)
    P_stack = np.empty((T, F, H), np.float32)
    projT = proj_w.T.astype(np.float64)
    for step in range(T):
        Wm = _gru_step(Wm, w_ih, w_hh, b_ih, b_hh)
        P_stack[step] = (Wm @ projT).astype(np.float32)

    # --- in-degree table C[v, tau] = #edges (k,v) with t_k <= tau ---
    flat = dst * T + t[src]
    hist = np.bincount(flat, minlength=N * T).astype(np.int32).reshape(N, T)
    Ccum = np.cumsum(hist, axis=1, dtype=np.int32)

    td = t[dst]
    active = t[src] <= td
    deg_dst = Ccum[dst, td] + 1
    deg_src = Ccum[src, td] + 1          # valid where active
    w_e = np.where(active,
                   1.0 / np.sqrt(deg_src.astype(np.float64) * deg_dst.astype(np.float64)),
                   0.0).astype(np.float32)
    sw = (1.0 / (Ccum[np.arange(N), t] + 1.0)).astype(np.float32)  # self weight

    # --- relabel nodes by (t, core, position) ---
    act_indeg = np.bincount(dst[active], minlength=N)
    order = np.argsort(t, kind="stable")          # grouped by t
    counts = np.bincount(t, minlength=T)
    starts = np.concatenate(([0], np.cumsum(counts)))[:-1]
    slot_core = np.empty(N, np.int32)
    slot_idx = np.empty(N, np.int32)
    orig_of = np.full((NCORES, NPAD), -1, np.int64)
    for tt in range(T):
        grp = order[starts[tt]: starts[tt] + counts[tt]]
        # ascending-degree round-robin: every core gets a near-identical
        # degree profile, so per-tile chunk counts align across cores
        grp = grp[np.argsort(act_indeg[grp], kind="stable")]
        for c in range(NCORES):
            seg = grp[c::NCORES]
            k = len(seg)
            assert k <= S, f"t-group {tt} core {c} has {k} > S={S} nodes"
            pos2 = np.arange(k)
            slot_core[seg] = c
            slot_idx[seg] = (tt * S + pos2).astype(np.int32)
            orig_of[c, tt * S + pos2] = seg

    # --- self rows, pre-scaled + transposed: xrT1 [128, NT*128], xrT2 [38, NT*128]
    xf = x.astype(np.float32)
    xrT1_cores, xrT2_cores = [], []
    for c in range(NCORES):
        ids = orig_of[c]
        valid = ids >= 0
        xr = np.zeros((NPAD, F), np.float32)
        xr[valid] = xf[ids[valid]] * sw[ids[valid]][:, None]
        xr3 = xr.reshape(NT_TILES, 128, F)
        xrT1_cores.append(np.ascontiguousarray(
            xr3[:, :, 0:F1].transpose(2, 0, 1).reshape(F1, NT_TILES * 128)
        ).astype(ml_dtypes.bfloat16))
        xrT2_cores.append(np.ascontiguousarray(
            xr3[:, :, F1:F].transpose(2, 0, 1).reshape(F2, NT_TILES * 128)
        ).astype(ml_dtypes.bfloat16))

    # --- per-core edge streams: split primary (first use of src) vs dup ---
    a_idx = np.nonzero(active)[0]
    e_src_a = src[a_idx]
    e_w_a = w_e[a_idx]
    e_core_a = slot_core[dst[a_idx]]
    e_slot_a = slot_idx[dst[a_idx]]

    x_bf = x.astype(ml_dtypes.bfloat16)
    prim = []            # per core: (src, w, slot) arrays for primary edges
    dups = []            # per core: (src, w, slot) arrays for dup edges
    prim_counts = np.zeros((NCORES, NT_TILES), np.int64)
    for c in range(NCORES):
        m = e_core_a == c
        s_c, w_c, sl_c = e_src_a[m], e_w_a[m], e_slot_a[m]
        o = np.argsort(sl_c, kind="stable")
        s_c, w_c, sl_c = s_c[o], w_c[o], sl_c[o]
        _, first_i = np.unique(s_c, return_index=True)
        is_prim = np.zeros(len(s_c), bool)
        is_prim[first_i] = True
        prim.append((s_c[is_prim], w_c[is_prim], sl_c[is_prim]))
        dups.append((s_c[~is_prim], w_c[~is_prim], sl_c[~is_prim]))
        prim_counts[c] = np.bincount(sl_c[is_prim] // 128, minlength=NT_TILES)

    klist = np.ceil(prim_counts.max(axis=0) / 128).astype(np.int64)
    col_base = np.concatenate(([0], np.cumsum(klist)))
    ECH = int(col_base[-1])

    # primary chunk tables + slab + first-use position of each source
    slab_cores, ewT_cores, elidT_cores = [], [], []
    fpos_cores = []
    for c in range(NCORES):
        s_c, w_c, sl_c = prim[c]
        ti_c = sl_c // 128
        rank = np.arange(len(s_c)) - np.concatenate(
            ([0], np.cumsum(np.bincount(ti_c, minlength=NT_TILES))))[:-1][ti_c]
        cidx = col_base[ti_c] + rank // 128
        part = rank % 128
        slab = np.zeros((128, ECH * F), ml_dtypes.bfloat16)
        slab[part[:, None], (cidx * F)[:, None] + np.arange(F)] = x_bf[s_c]
        ewT = np.zeros((128, ECH), np.float32)
        elidT = np.zeros((128, ECH), np.int64)
        ewT[part, cidx] = w_c
        elidT[part, cidx] = sl_c % 128
        slab_cores.append(slab)
        ewT_cores.append(ewT)
        elidT_cores.append(elidT)
        # dup gather row index into slab viewed as [128*ECH, F]: p*ECH + cidx
        fpos = dict(zip(s_c.tolist(), (part.astype(np.int64) * ECH + cidx).tolist()))
        fpos_cores.append(fpos)

    # --- dup chunks: per t-group, sorted by slot, chunked by 128 (SPMD-common) ---
    dup_by_gc = [[None] * NCORES for _ in range(T)]
    dg_counts = np.zeros((NCORES, T), np.int64)
    for c in range(NCORES):
        s_c, w_c, sl_c = dups[c]
        g_c = sl_c // S
        o = np.lexsort((sl_c, g_c))
        s_c, w_c, sl_c, g_c = s_c[o], w_c[o], sl_c[o], g_c[o]
        for g in range(T):
            m = g_c == g
            dup_by_gc[g][c] = (s_c[m], w_c[m], sl_c[m])
            dg_counts[c, g] = m.sum()
    DG = dg_counts.max(axis=0)
    dch_group = []       # group of each dup chunk
    dch_spans = []       # tuple of ti values per dup chunk
    for g in range(T):
        for k in range(int(np.ceil(DG[g] / 128))):
            span = set()
            for c in range(NCORES):
                sl = dup_by_gc[g][c][2][k * 128:(k + 1) * 128]
                span.update((sl // 128).tolist())
            if not span:
                continue
            dch_group.append((g, k))
            dch_spans.append(tuple(sorted(int(v) for v in span)))
    NDCH = len(dch_group)
    NSPAN = sum(len(s) for s in dch_spans)

    dupidx_cores = np.zeros((NCORES, 128, max(NDCH, 1)), np.int32)
    ewd_cores = np.zeros((NCORES, 128, max(NSPAN, 1)), np.float32)
    elidd_cores = np.zeros((NCORES, 128, max(NSPAN, 1)), np.int64)
    spi = 0
    for d in range(NDCH):
        g, k = dch_group[d]
        for c in range(NCORES):
            s_c, w_c, sl_c = dup_by_gc[g][c]
            s_k = s_c[k * 128:(k + 1) * 128]
            w_k = w_c[k * 128:(k + 1) * 128]
            sl_k = sl_c[k * 128:(k + 1) * 128]
            nk = len(s_k)
            fp = fpos_cores[c]
            if nk:
                dupidx_cores[c, :nk, d] = [fp[int(sv)] for sv in s_k]
            for si, ti in enumerate(dch_spans[d]):
                mspan = (sl_k // 128) == ti
                col = spi + si
                ewd_cores[c, :nk, col] = np.where(mspan, w_k, 0.0)
                elidd_cores[c, :nk, col] = np.where(mspan, sl_k % 128, 0)
        spi += len(dch_spans[d])

    # ---- pack dense one-hot stream in device consumption order ----
    ti_spans_h = [[] for _ in range(NT_TILES)]
    spi = 0
    for d in range(NDCH):
        for si, ti in enumerate(dch_spans[d]):
            ti_spans_h[ti].append(spi + si)
        spi += len(dch_spans[d])
    order_cols = []          # (kind, col): kind 0 = primary cidx, 1 = span col
    col_base_h = np.concatenate(([0], np.cumsum(klist)))
    for ti in range(NT_TILES):
        for k in range(int(klist[ti])):
            order_cols.append((0, int(col_base_h[ti]) + k))
        for spcol in ti_spans_h[ti]:
            order_cols.append((1, spcol))
    NOH = len(order_cols)
    oh_cores = []
    rows = np.arange(128)
    for c in range(NCORES):
        ohs = np.zeros((128, NOH * 128), ml_dtypes.bfloat16)
        for q, (kind, col) in enumerate(order_cols):
            if kind == 0:
                wv, lid = ewT_cores[c][:, col], elidT_cores[c][:, col]
            else:
                wv, lid = ewd_cores[c][:, col], elidd_cores[c][:, col]
            ohs[rows, q * 128 + lid] = wv
        oh_cores.append(ohs)

    per_core = []
    for c in range(NCORES):
        per_core.append({
            "slab": slab_cores[c],
            "xrT1": xrT1_cores[c],
            "xrT2": xrT2_cores[c],
            "ohs": oh_cores[c],
            "dupidx": dupidx_cores[c],
            "P_stack": np.ascontiguousarray(
                P_stack.transpose(1, 0, 2).reshape(F, T * H)).astype(ml_dtypes.bfloat16),
            "projb": proj_b.reshape(H, 1).astype(np.float32),
            "clsw": cls_w.T.astype(ml_dtypes.bfloat16).copy(),   # [H, C]
        })
    K = (tuple(int(v) for v in klist), tuple(dch_spans))
    return per_core, orig_of, K


def _build(K):
    import concourse.bacc as bacc
    import concourse.bass as bass
    import concourse.mybir as mybir
    import concourse.tile as tile

    klist, dch_spans = K
    klist = list(klist)
    col_base = [0]
    for v in klist:
        col_base.append(col_base[-1] + v)
    ECH = col_base[-1]
    NDCH = len(dch_spans)
    NSPAN = sum(len(s) for s in dch_spans)
    ti_spans = [[] for _ in range(NT_TILES)]
    spi = 0
    for d in range(NDCH):
        for si, ti in enumerate(dch_spans[d]):
            ti_spans[ti].append((d, spi + si))
        spi += len(dch_spans[d])
    # oh stream slot of each (kind, col), mirroring host packing order
    oh_of = {}
    q = 0
    for ti in range(NT_TILES):
        for k in range(klist[ti]):
            oh_of[(0, col_base[ti] + k)] = q
            q += 1
        for d, spcol in ti_spans[ti]:
            oh_of[(1, spcol)] = q
            q += 1

    nc = bacc.Bacc("TRN2", target_bir_lowering=False, debug=False,
                   num_devices=NCORES)
    dt = mybir.dt.float32
    bf = mybir.dt.bfloat16
    slab_d = nc.dram_tensor("slab", [128, ECH * F], bf, kind="ExternalInput")
    xrT1_d = nc.dram_tensor("xrT1", [F1, NT_TILES * 128], bf, kind="ExternalInput")
    xrT2_d = nc.dram_tensor("xrT2", [F2, NT_TILES * 128], bf, kind="ExternalInput")
    NOH = ECH + NSPAN
    ohs_d = nc.dram_tensor("ohs", [128, NOH * 128], bf, kind="ExternalInput")
    dupidx_d = nc.dram_tensor("dupidx", [128, max(NDCH, 1)], mybir.dt.int32,
                              kind="ExternalInput")
    P_d = nc.dram_tensor("P_stack", [F, T * H], bf, kind="ExternalInput")
    projb_d = nc.dram_tensor("projb", [H, 1], dt, kind="ExternalInput")
    clsw_d = nc.dram_tensor("clsw", [H, C], bf, kind="ExternalInput")
    lgT_d = nc.dram_tensor("lgT", [C, NPAD], dt, kind="ExternalOutput")

    def pieces(total, first, big):
        bounds = [0]
        while bounds[-1] < total:
            step = first if len(bounds) <= 4 else big
            bounds.append(min(total, bounds[-1] + step))
        return bounds

    slab_b = pieces(ECH, 8, 48)
    piece_of_chunk = [0] * ECH
    for pi in range(len(slab_b) - 1):
        for cdx in range(slab_b[pi], slab_b[pi + 1]):
            piece_of_chunk[cdx] = pi
    xrt_b = pieces(NT_TILES, 10, 50)
    piece_of_tile = [0] * NT_TILES
    for pi in range(len(xrt_b) - 1):
        for tix in range(xrt_b[pi], xrt_b[pi + 1]):
            piece_of_tile[tix] = pi

    with tile.TileContext(nc) as tc:
        with (
            tc.tile_pool(name="const", bufs=1) as cpool,
            tc.tile_pool(name="meta", bufs=1) as mpool,
            tc.tile_pool(name="slab", bufs=3) as slabpool,
            tc.tile_pool(name="ohslab", bufs=2) as ohslabpool,
            tc.tile_pool(name="xrt", bufs=2) as xrtpool,
            tc.tile_pool(name="yd", bufs=1) as ydpool,
            tc.tile_pool(name="oh", bufs=12) as ohpool,
            tc.tile_pool(name="st", bufs=3) as stpool,
            tc.tile_pool(name="zt", bufs=3) as ztpool,
            tc.tile_pool(name="lg", bufs=1) as lgpool,
            tc.tile_pool(name="ps", bufs=3, space="PSUM") as pspool,
            tc.tile_pool(name="ps2", bufs=3, space="PSUM") as ps2pool,
            tc.tile_pool(name="pz", bufs=2, space="PSUM") as pzpool,
        ):
            projb_sb = cpool.tile([H, 1], dt)
            nc.sync.dma_start(out=projb_sb[:], in_=projb_d[:])
            clsw_sb = cpool.tile([H, C], bf)
            nc.sync.dma_start(out=clsw_sb[:], in_=clsw_d[:])
            dupidx_sb = mpool.tile([128, max(NDCH, 1)], mybir.dt.int32)
            nc.sync.dma_start(out=dupidx_sb[:], in_=dupidx_d[:])
            # dup-chunk gathers (from the DRAM slab, row view [128*ECH, F])
            slab_rows = slab_d[:].rearrange("p (c f) -> (p c) f", f=F)
            ydup = []
            for d in range(NDCH):
                y = ydpool.tile([128, F], bf, tag=f"yd{d}")
                nc.gpsimd.indirect_dma_start(
                    out=y[:], out_offset=None, in_=slab_rows,
                    in_offset=bass.IndirectOffsetOnAxis(
                        ap=dupidx_sb[:, d:d + 1], axis=0),
                )
                ydup.append(y)

            oh_b = pieces(NOH, 10, 72)
            piece_of_oh = [0] * NOH
            for pi in range(len(oh_b) - 1):
                for qx in range(oh_b[pi], oh_b[pi + 1]):
                    piece_of_oh[qx] = pi
            OHW = max(oh_b[i + 1] - oh_b[i] for i in range(len(oh_b) - 1))
            SLW = max(slab_b[i + 1] - slab_b[i] for i in range(len(slab_b) - 1))
            XRW = max(xrt_b[i + 1] - xrt_b[i] for i in range(len(xrt_b) - 1))

            # interleave initial pieces so tile 0's inputs land first
            ohslabs, slabs, xrt1s, xrt2s = [], [], [], []
            nmax = max(len(oh_b), len(slab_b), len(xrt_b)) - 1
            for pi in range(nmax):
                if pi < len(slab_b) - 1:
                    c0, c1 = slab_b[pi], slab_b[pi + 1]
                    stile = slabpool.tile([128, SLW * F], bf, tag="slab")
                    nc.sync.dma_start(out=stile[:, 0:(c1 - c0) * F],
                                      in_=slab_d[:, c0 * F:c1 * F])
                    slabs.append(stile)
                if pi < len(oh_b) - 1:
                    c0, c1 = oh_b[pi], oh_b[pi + 1]
                    otile = ohslabpool.tile([128, OHW * 128], bf, tag="ohslab")
                    nc.sync.dma_start(out=otile[:, 0:(c1 - c0) * 128],
                                      in_=ohs_d[:, c0 * 128:c1 * 128])
                    ohslabs.append(otile)
                if pi < len(xrt_b) - 1:
                    c0, c1 = xrt_b[pi], xrt_b[pi + 1]
                    x1 = xrtpool.tile([F1, XRW * 128], bf, tag="xrt1")
                    nc.sync.dma_start(out=x1[:, 0:(c1 - c0) * 128],
                                      in_=xrT1_d[:, c0 * 128:c1 * 128])
                    x2 = xrtpool.tile([F2, XRW * 128], bf, tag="xrt2")
                    nc.sync.dma_start(out=x2[:, 0:(c1 - c0) * 128],
                                      in_=xrT2_d[:, c0 * 128:c1 * 128])
                    xrt1s.append(x1)
                    xrt2s.append(x2)

            P1_sb = mpool.tile([F1, T * H], bf)
            P2_sb = mpool.tile([F2, T * H], bf)
            for pp in range(4):
                nc.sync.dma_start(out=P1_sb[:, pp * 1568:(pp + 1) * 1568],
                                  in_=P_d[0:F1, pp * 1568:(pp + 1) * 1568])
                nc.sync.dma_start(out=P2_sb[:, pp * 1568:(pp + 1) * 1568],
                                  in_=P_d[F1:F, pp * 1568:(pp + 1) * 1568])

            for ti in range(NT_TILES):
                tt = ti // TILES_PER_T
                j = ti % TILES_PER_T
                kti = klist[ti]
                spans = ti_spans[ti]
                nmm = kti + len(spans)
                xpi = piece_of_tile[ti]
                x1 = xrt1s[xpi]
                x2 = xrt2s[xpi]
                xo = (ti - xrt_b[xpi]) * 128
                if j == 0:
                    sT1q = stpool.tile([128, S], bf, tag="sT1q")
                    sT2q = stpool.tile([128, S], bf, tag="sT2q")
                if nmm == 0:
                    nc.vector.tensor_copy(out=sT1q[:, j * 128:(j + 1) * 128],
                                          in_=x1[:, xo:xo + 128])
                    nc.scalar.copy(out=sT2q[0:F2, j * 128:(j + 1) * 128],
                                   in_=x2[:, xo:xo + 128])
                else:
                    psum_s = pspool.tile([128, 128], dt, space="PSUM")
                    psum_s2 = ps2pool.tile([F2, 128], dt, space="PSUM")
                    # ops: (lhsT source, oh stream slot) per chunk, F1 pass then F2
                    ops = []
                    for k in range(kti):
                        cidx = col_base[ti] + k
                        spi_ = piece_of_chunk[cidx]
                        ysl = slabs[spi_]
                        off = (cidx - slab_b[spi_]) * F
                        q = oh_of[(0, cidx)]
                        ops.append((ysl, off, q))
                    for d, spcol in spans:
                        ops.append((ydup[d], 0, oh_of[(1, spcol)]))
                    for i, (ysl, off, q) in enumerate(ops):
                        opi = piece_of_oh[q]
                        ohsl = ohslabs[opi]
                        oho = (q - oh_b[opi]) * 128
                        nc.tensor.matmul(out=psum_s[:], lhsT=ysl[:, off:off + F1],
                                         rhs=ohsl[:, oho:oho + 128],
                                         start=i == 0, stop=i == len(ops) - 1)
                    for i, (ysl, off, q) in enumerate(ops):
                        opi = piece_of_oh[q]
                        ohsl = ohslabs[opi]
                        oho = (q - oh_b[opi]) * 128
                        nc.tensor.matmul(out=psum_s2[:], lhsT=ysl[:, off + F1:off + F],
                                         rhs=ohsl[:, oho:oho + 128],
                                         start=i == 0, stop=i == len(ops) - 1)
                    # merge self term during PSUM -> SBUF copy
                    nc.vector.tensor_tensor(
                        out=sT1q[:, j * 128:(j + 1) * 128], in0=psum_s[:],
                        in1=x1[:, xo:xo + 128], op=mybir.AluOpType.add)
                    nc.vector.tensor_tensor(
                        out=sT2q[0:F2, j * 128:(j + 1) * 128], in0=psum_s2[:],
                        in1=x2[:, xo:xo + 128], op=mybir.AluOpType.add)
                if j == TILES_PER_T - 1:
                    # ---- stage 2 batched: z^T = relu(P_t^T s^T + b)
                    pz_a = pzpool.tile([128, 512], dt, space="PSUM", tag="pz")
                    pz_b = pzpool.tile([128, S - 512], dt, space="PSUM", tag="pz")
                    pc0 = tt * H
                    nc.tensor.matmul(out=pz_a[:], lhsT=P1_sb[:, pc0:pc0 + H],
                                     rhs=sT1q[:, 0:512], start=True, stop=False)
                    nc.tensor.matmul(out=pz_b[:], lhsT=P1_sb[:, pc0:pc0 + H],
                                     rhs=sT1q[:, 512:S], start=True, stop=False)
                    nc.tensor.matmul(out=pz_a[:], lhsT=P2_sb[:, pc0:pc0 + H],
                                     rhs=sT2q[0:F2, 0:512], start=False, stop=True)
                    nc.tensor.matmul(out=pz_b[:], lhsT=P2_sb[:, pc0:pc0 + H],
                                     rhs=sT2q[0:F2, 512:S], start=False, stop=True)
                    zTq = ztpool.tile([128, S], bf, tag="zTq")
                    nc.scalar.activation(out=zTq[:, 0:512], in_=pz_a[:],
                                         func=mybir.ActivationFunctionType.Relu,
                                         bias=projb_sb[:, 0:1])
                    nc.scalar.activation(out=zTq[:, 512:S], in_=pz_b[:],
                                         func=mybir.ActivationFunctionType.Relu,
                                         bias=projb_sb[:, 0:1])
                    # ---- stage 3 batched: lg^T for the whole group
                    if tt % GBATCH == 0:
                        lgb = lgpool.tile([C, GBATCH * S], dt, tag="lgb")
                    lo = (tt % GBATCH) * S
                    psum_lg = pzpool.tile([C, 512], dt, space="PSUM", tag="pz")
                    nc.tensor.matmul(out=psum_lg[:], lhsT=clsw_sb[:],
                                     rhs=zTq[:, 0:512], start=True, stop=True)
                    nc.scalar.copy(out=lgb[:, lo:lo + 512], in_=psum_lg[:])
                    psum_lg2 = pzpool.tile([C, 512], dt, space="PSUM", tag="pz")
                    nc.tensor.matmul(out=psum_lg2[:, 0:S - 512], lhsT=clsw_sb[:],
                                     rhs=zTq[:, 512:S], start=True, stop=True)
                    nc.scalar.copy(out=lgb[:, lo + 512:lo + S],
                                   in_=psum_lg2[:, 0:S - 512])
                    if tt % GBATCH == GBATCH - 1 or tt == T - 1:
                        b0 = (tt - tt % GBATCH) * S
                        nc.sync.dma_start(
                            out=lgT_d[:, b0:b0 + (tt % GBATCH) * S + S],
                            in_=lgb[:, 0:(tt % GBATCH) * S + S])
    nc.compile()
    return nc


def kernel(**inputs):
    from concourse.bass_utils import run_bass_kernel_spmd

    np_inputs = {k: np.asarray(v) for k, v in inputs.items()}
    per_core, orig_of, K = _host_prep(**np_inputs)

    if K not in _cache:
        _cache[K] = _build(K)
    nc = _cache[K]

    res = run_bass_kernel_spmd(nc, per_core, list(range(NCORES)))

    cls_b = np_inputs["cls_b"].astype(np.float32)
    logits = np.zeros((N, C), np.float32)
    for c in range(NCORES):
        ids = orig_of[c]
        valid = ids >= 0
        lgT = res.results[c]["lgT"]                    # [C, NPAD]
        logits[ids[valid]] = lgT.T[valid]
    logits += cls_b
    return logits


# revision 18
# speedup vs baseline: 1.1716x; 1.0390x over previous
"""EvolveGCN-O kernel for Trainium2 (8 NeuronCores).

Key algebraic restructure: the reference keeps, for node i, only the logits
computed at timestep t_i = time_step[i].  The GCN aggregation at time t is
linear in x, so

  logits_i = cls( relu( (sum_{j->i active@t_i} norm_ji x_j + x_i/deg_i) @ W_{t_i} @ proj^T + b ) )

with norm/deg computed from in-degree counts at t_i.  So instead of 49 full
GCN passes we do ONE edge-aggregation pass (over edges (j,i) with
t_j <= t_i) and one per-timestep-group matmul with P_t = W_t @ proj^T.

Sharding (METIS-style partition + halo exchange): nodes are partitioned
across 8 cores by (t, core); each core receives the deduplicated "halo" set
of x rows its edges reference, laid out in first-use order so the edge
aggregation streams it SEQUENTIALLY (no per-row descriptors).  Only repeated
sources (~9% of edges) are fetched by on-device indirect gathers.

Device work per core:
  stage 1: s^T accumulation: psum += slab_chunk^T @ onehot(dst slot, w_e)
           for primary edges; per-group indirect-gathered dup chunks add the
           repeated-source edges; the self term (sw_i * x_i)^T is streamed
           pre-transposed and merged during the PSUM->SBUF copy (DVE add).
  stage 2: z^T = relu(P_t^T s^T + b)   (t static per tile group)
  stage 3: lg^T = cls_w^T^T z^T, stores batched over 7 t-groups
Host does: GRU weight evolution (tiny FxF chain), degree tables, edge
weights, graph partitioning / relabeling / halo tables, unpermute + cls bias.
"""

import ml_dtypes
import numpy as np

N, E, F, H, C, T = 200000, 500000, 166, 128, 2, 49
NCORES = 8
S = 640                      # per-core slots per timestep group (5 tiles)
TILES_PER_T = S // 128       # 5
NT_TILES = T * TILES_PER_T   # 245
NPAD = T * S                 # 31360 slots per core
F1 = 128                     # feature chunk 1
F2 = F - F1                  # 38
GBATCH = 7                   # t-groups per output store

_cache = {}


def _gru_step(Wm, w_ih, w_hh, b_ih, b_hh):
    gi = Wm @ w_ih.T + b_ih
    gh = Wm @ w_hh.T + b_hh
    i_r, i_z, i_n = np.split(gi, 3, axis=-1)
    h_r, h_z, h_n = np.split(gh, 3, axis=-1)
    r = 1.0 / (1.0 + np.exp(-(i_r + h_r)))
    z = 1.0 / (1.0 + np.exp(-(i_z + h_z)))
    nn_ = np.tanh(i_n + r * h_n)
    return (1.0 - z) * nn_ + z * Wm


def _host_prep(x, edge_index, time_step, initial_w, gru_w_ih, gru_w_hh,
               gru_b_ih, gru_b_hh, proj_w, proj_b, cls_w, cls_b):
    src = edge_index[0].astype(np.int64)
    dst = edge_index[1].astype(np.int64)
    t = time_step.astype(np.int64)

    # --- evolve W, fuse with proj ---
    Wm = initial_w.astype(np.float64)
    w_ih = gru_w_ih.astype(np.float64)
    w_hh = gru_w_hh.astype(np.float64)
    b_ih = gru_b_ih.astype(np.float64)
    b_hh = gru_b_hh.astype(np.float64)
    P_stack = np.empty((T, F, H), np.float32)
    projT = proj_w.T.astype(np.float64)
    for step in range(T):
        Wm = _gru_step(Wm, w_ih, w_hh, b_ih, b_hh)
        P_stack[step] = (Wm @ projT).astype(np.float32)

    # --- in-degree table C[v, tau] = #edges (k,v) with t_k <= tau ---
    flat = dst * T + t[src]
    hist = np.bincount(flat, minlength=N * T).astype(np.int32).reshape(N, T)
    Ccum = np.cumsum(hist, axis=1, dtype=np.int32)

    td = t[dst]
    active = t[src] <= td
    deg_dst = Ccum[dst, td] + 1
    deg_src = Ccum[src, td] + 1          # valid where active
    w_e = np.where(active,
                   1.0 / np.sqrt(deg_src.astype(np.float64) * deg_dst.astype(np.float64)),
                   0.0).astype(np.float32)
    sw = (1.0 / (Ccum[np.arange(N), t] + 1.0)).astype(np.float32)  # self weight

    # --- relabel nodes by (t, core, position) ---
    act_indeg = np.bincount(dst[active], minlength=N)
    order = np.argsort(t, kind="stable")          # grouped by t
    counts = np.bincount(t, minlength=T)
    starts = np.concatenate(([0], np.cumsum(counts)))[:-1]
    slot_core = np.empty(N, np.int32)
    slot_idx = np.empty(N, np.int32)
    orig_of = np.full((NCORES, NPAD), -1, np.int64)
    for tt in range(T):
        grp = order[starts[tt]: starts[tt] + counts[tt]]
        # ascending-degree round-robin: every core gets a near-identical
        # degree profile, so per-tile chunk counts align across cores
        grp = grp[np.argsort(act_indeg[grp], kind="stable")]
        for c in range(NCORES):
            seg = grp[c::NCORES]
            k = len(seg)
            assert k <= S, f"t-group {tt} core {c} has {k} > S={S} nodes"
            pos2 = np.arange(k)
            slot_core[seg] = c
            slot_idx[seg] = (tt * S + pos2).astype(np.int32)
            orig_of[c, tt * S + pos2] = seg

    # --- self rows, pre-scaled + transposed: xrT1 [128, NT*128], xrT2 [38, NT*128]
    xf = x.astype(np.float32)
    xrT1_cores, xrT2_cores = [], []
    for c in range(NCORES):
        ids = orig_of[c]
        valid = ids >= 0
        xr = np.zeros((NPAD, F), np.float32)
        xr[valid] = xf[ids[valid]] * sw[ids[valid]][:, None]
        xr3 = xr.reshape(NT_TILES, 128, F)
        xrT1_cores.append(np.ascontiguousarray(
            xr3[:, :, 0:F1].transpose(2, 0, 1).reshape(F1, NT_TILES * 128)
        ).astype(ml_dtypes.bfloat16))
        xrT2_cores.append(np.ascontiguousarray(
            xr3[:, :, F1:F].transpose(2, 0, 1).reshape(F2, NT_TILES * 128)
        ).astype(ml_dtypes.bfloat16))

    # --- per-core edge streams: split primary (first use of src) vs dup ---
    a_idx = np.nonzero(active)[0]
    e_src_a = src[a_idx]
    e_w_a = w_e[a_idx]
    e_core_a = slot_core[dst[a_idx]]
    e_slot_a = slot_idx[dst[a_idx]]

    x_bf = x.astype(ml_dtypes.bfloat16)
    prim = []            # per core: (src, w, slot) arrays for primary edges
    dups = []            # per core: (src, w, slot) arrays for dup edges
    prim_counts = np.zeros((NCORES, NT_TILES), np.int64)
    for c in range(NCORES):
        m = e_core_a == c
        s_c, w_c, sl_c = e_src_a[m], e_w_a[m], e_slot_a[m]
        o = np.argsort(sl_c, kind="stable")
        s_c, w_c, sl_c = s_c[o], w_c[o], sl_c[o]
        _, first_i = np.unique(s_c, return_index=True)
        is_prim = np.zeros(len(s_c), bool)
        is_prim[first_i] = True
        prim.append((s_c[is_prim], w_c[is_prim], sl_c[is_prim]))
        dups.append((s_c[~is_prim], w_c[~is_prim], sl_c[~is_prim]))
        prim_counts[c] = np.bincount(sl_c[is_prim] // 128, minlength=NT_TILES)

    klist = np.ceil(prim_counts.max(axis=0) / 128).astype(np.int64)
    col_base = np.concatenate(([0], np.cumsum(klist)))
    ECH = int(col_base[-1])

    # primary chunk tables + slab + first-use position of each source
    slab_cores, ewT_cores, elidT_cores = [], [], []
    fpos_cores = []
    for c in range(NCORES):
        s_c, w_c, sl_c = prim[c]
        ti_c = sl_c // 128
        rank = np.arange(len(s_c)) - np.concatenate(
            ([0], np.cumsum(np.bincount(ti_c, minlength=NT_TILES))))[:-1][ti_c]
        cidx = col_base[ti_c] + rank // 128
        part = rank % 128
        slab = np.zeros((128, ECH * F), ml_dtypes.bfloat16)
        slab[part[:, None], (cidx * F)[:, None] + np.arange(F)] = x_bf[s_c]
        ewT = np.zeros((128, ECH), np.float32)
        elidT = np.zeros((128, ECH), np.int64)
        ewT[part, cidx] = w_c
        elidT[part, cidx] = sl_c % 128
        slab_cores.append(slab)
        ewT_cores.append(ewT)
        elidT_cores.append(elidT)
        # dup gather row index into slab viewed as [128*ECH, F]: p*ECH + cidx
        fpos = dict(zip(s_c.tolist(), (part.astype(np.int64) * ECH + cidx).tolist()))
        fpos_cores.append(fpos)

    # --- dup chunks: per t-group, sorted by slot, chunked by 128 (SPMD-common) ---
    dup_by_gc = [[None] * NCORES for _ in range(T)]
    dg_counts = np.zeros((NCORES, T), np.int64)
    for c in range(NCORES):
        s_c, w_c, sl_c = dups[c]
        g_c = sl_c // S
        o = np.lexsort((sl_c, g_c))
        s_c, w_c, sl_c, g_c = s_c[o], w_c[o], sl_c[o], g_c[o]
        for g in range(T):
            m = g_c == g
            dup_by_gc[g][c] = (s_c[m], w_c[m], sl_c[m])
            dg_counts[c, g] = m.sum()
    DG = dg_counts.max(axis=0)
    dch_group = []       # group of each dup chunk
    dch_spans = []       # tuple of ti values per dup chunk
    for g in range(T):
        for k in range(int(np.ceil(DG[g] / 128))):
            span = set()
            for c in range(NCORES):
                sl = dup_by_gc[g][c][2][k * 128:(k + 1) * 128]
                span.update((sl // 128).tolist())
            if not span:
                continue
            dch_group.append((g, k))
            dch_spans.append(tuple(sorted(int(v) for v in span)))
    NDCH = len(dch_group)
    NSPAN = sum(len(s) for s in dch_spans)

    dupidx_cores = np.zeros((NCORES, 128, max(NDCH, 1)), np.int32)
    ewd_cores = np.zeros((NCORES, 128, max(NSPAN, 1)), np.float32)
    elidd_cores = np.zeros((NCORES, 128, max(NSPAN, 1)), np.int64)
    spi = 0
    for d in range(NDCH):
        g, k = dch_group[d]
        for c in range(NCORES):
            s_c, w_c, sl_c = dup_by_gc[g][c]
            s_k = s_c[k * 128:(k + 1) * 128]
            w_k = w_c[k * 128:(k + 1) * 128]
            sl_k = sl_c[k * 128:(k + 1) * 128]
            nk = len(s_k)
            fp = fpos_cores[c]
            if nk:
                dupidx_cores[c, :nk, d] = [fp[int(sv)] for sv in s_k]
            for si, ti in enumerate(dch_spans[d]):
                mspan = (sl_k // 128) == ti
                col = spi + si
                ewd_cores[c, :nk, col] = np.where(mspan, w_k, 0.0)
                elidd_cores[c, :nk, col] = np.where(mspan, sl_k % 128, 0)
        spi += len(dch_spans[d])

    # ---- pack dense one-hot stream in device consumption order ----
    ti_spans_h = [[] for _ in range(NT_TILES)]
    spi = 0
    for d in range(NDCH):
        for si, ti in enumerate(dch_spans[d]):
            ti_spans_h[ti].append(spi + si)
        spi += len(dch_spans[d])
    order_cols = []          # (kind, col): kind 0 = primary cidx, 1 = span col
    col_base_h = np.concatenate(([0], np.cumsum(klist)))
    for ti in range(NT_TILES):
        for k in range(int(klist[ti])):
            order_cols.append((0, int(col_base_h[ti]) + k))
        for spcol in ti_spans_h[ti]:
            order_cols.append((1, spcol))
    NOH = len(order_cols)
    oh_cores = []
    rows = np.arange(128)
    for c in range(NCORES):
        ohs = np.zeros((128, NOH * 128), ml_dtypes.bfloat16)
        for q, (kind, col) in enumerate(order_cols):
            if kind == 0:
                wv, lid = ewT_cores[c][:, col], elidT_cores[c][:, col]
            else:
                wv, lid = ewd_cores[c][:, col], elidd_cores[c][:, col]
            ohs[rows, q * 128 + lid] = wv
        oh_cores.append(ohs)

    per_core = []
    for c in range(NCORES):
        per_core.append({
            "slab": slab_cores[c],
            "xrT1": xrT1_cores[c],
            "xrT2": xrT2_cores[c],
            "ohs": oh_cores[c],
            "dupidx": dupidx_cores[c],
            "P_stack": np.ascontiguousarray(
                P_stack.transpose(1, 0, 2).reshape(F, T * H)).astype(ml_dtypes.bfloat16),
            "projb": proj_b.reshape(H, 1).astype(np.float32),
            "clsw": cls_w.T.astype(ml_dtypes.bfloat16).copy(),   # [H, C]
        })
    K = (tuple(int(v) for v in klist), tuple(dch_spans))
    return per_core, orig_of, K


def _build(K):
    import concourse.bacc as bacc
    import concourse.bass as bass
    import concourse.mybir as mybir
    import concourse.tile as tile

    klist, dch_spans = K
    klist = list(klist)
    col_base = [0]
    for v in klist:
        col_base.append(col_base[-1] + v)
    ECH = col_base[-1]
    NDCH = len(dch_spans)
    NSPAN = sum(len(s) for s in dch_spans)
    ti_spans = [[] for _ in range(NT_TILES)]
    spi = 0
    for d in range(NDCH):
        for si, ti in enumerate(dch_spans[d]):
            ti_spans[ti].append((d, spi + si))
        spi += len(dch_spans[d])
    # oh stream slot of each (kind, col), mirroring host packing order
    oh_of = {}
    q = 0
    for ti in range(NT_TILES):
        for k in range(klist[ti]):
            oh_of[(0, col_base[ti] + k)] = q
            q += 1
        for d, spcol in ti_spans[ti]:
            oh_of[(1, spcol)] = q
            q += 1

    nc = bacc.Bacc("TRN2", target_bir_lowering=False, debug=False,
                   num_devices=NCORES)
    dt = mybir.dt.float32
    bf = mybir.dt.bfloat16
    slab_d = nc.dram_tensor("slab", [128, ECH * F], bf, kind="ExternalInput")
    xrT1_d = nc.dram_tensor("xrT1", [F1, NT_TILES * 128], bf, kind="ExternalInput")
    xrT2_d = nc.dram_tensor("xrT2", [F2, NT_TILES * 128], bf, kind="ExternalInput")
    NOH = ECH + NSPAN
    ohs_d = nc.dram_tensor("ohs", [128, NOH * 128], bf, kind="ExternalInput")
    dupidx_d = nc.dram_tensor("dupidx", [128, max(NDCH, 1)], mybir.dt.int32,
                              kind="ExternalInput")
    P_d = nc.dram_tensor("P_stack", [F, T * H], bf, kind="ExternalInput")
    projb_d = nc.dram_tensor("projb", [H, 1], dt, kind="ExternalInput")
    clsw_d = nc.dram_tensor("clsw", [H, C], bf, kind="ExternalInput")
    lgT_d = nc.dram_tensor("lgT", [C, NPAD], dt, kind="ExternalOutput")

    def pieces(total, first, big):
        bounds = [0]
        while bounds[-1] < total:
            step = first if len(bounds) <= 4 else big
            bounds.append(min(total, bounds[-1] + step))
        return bounds

    slab_b = pieces(ECH, 8, 24)
    piece_of_chunk = [0] * ECH
    for pi in range(len(slab_b) - 1):
        for cdx in range(slab_b[pi], slab_b[pi + 1]):
            piece_of_chunk[cdx] = pi
    xrt_b = pieces(NT_TILES, 10, 30)
    piece_of_tile = [0] * NT_TILES
    for pi in range(len(xrt_b) - 1):
        for tix in range(xrt_b[pi], xrt_b[pi + 1]):
            piece_of_tile[tix] = pi

    with tile.TileContext(nc) as tc:
        with (
            tc.tile_pool(name="const", bufs=1) as cpool,
            tc.tile_pool(name="meta", bufs=1) as mpool,
            tc.tile_pool(name="slab", bufs=3) as slabpool,
            tc.tile_pool(name="ohslab", bufs=2) as ohslabpool,
            tc.tile_pool(name="xrt", bufs=2) as xrtpool,
            tc.tile_pool(name="yd", bufs=1) as ydpool,
            tc.tile_pool(name="oh", bufs=12) as ohpool,
            tc.tile_pool(name="st", bufs=3) as stpool,
            tc.tile_pool(name="zt", bufs=3) as ztpool,
            tc.tile_pool(name="lg", bufs=1) as lgpool,
            tc.tile_pool(name="ps", bufs=3, space="PSUM") as pspool,
            tc.tile_pool(name="ps2", bufs=3, space="PSUM") as ps2pool,
            tc.tile_pool(name="pz", bufs=2, space="PSUM") as pzpool,
        ):
            projb_sb = cpool.tile([H, 1], dt)
            nc.sync.dma_start(out=projb_sb[:], in_=projb_d[:])
            clsw_sb = cpool.tile([H, C], bf)
            nc.sync.dma_start(out=clsw_sb[:], in_=clsw_d[:])
            dupidx_sb = mpool.tile([128, max(NDCH, 1)], mybir.dt.int32)
            nc.sync.dma_start(out=dupidx_sb[:], in_=dupidx_d[:])
            # dup-chunk gathers (from the DRAM slab, row view [128*ECH, F])
            slab_rows = slab_d[:].rearrange("p (c f) -> (p c) f", f=F)
            ydup = []
            for d in range(NDCH):
                y = ydpool.tile([128, F], bf, tag=f"yd{d}")
                nc.gpsimd.indirect_dma_start(
                    out=y[:], out_offset=None, in_=slab_rows,
                    in_offset=bass.IndirectOffsetOnAxis(
                        ap=dupidx_sb[:, d:d + 1], axis=0),
                )
                ydup.append(y)

            oh_b = pieces(NOH, 10, 36)
            piece_of_oh = [0] * NOH
            for pi in range(len(oh_b) - 1):
                for qx in range(oh_b[pi], oh_b[pi + 1]):
                    piece_of_oh[qx] = pi
            OHW = max(oh_b[i + 1] - oh_b[i] for i in range(len(oh_b) - 1))
            SLW = max(slab_b[i + 1] - slab_b[i] for i in range(len(slab_b) - 1))
            XRW = max(xrt_b[i + 1] - xrt_b[i] for i in range(len(xrt_b) - 1))

            # interleave initial pieces so tile 0's inputs land first
            ohslabs, slabs, xrt1s, xrt2s = [], [], [], []
            nmax = max(len(oh_b), len(slab_b), len(xrt_b)) - 1
            for pi in range(nmax):
                if pi < len(slab_b) - 1:
                    c0, c1 = slab_b[pi], slab_b[pi + 1]
                    stile = slabpool.tile([128, SLW * F], bf, tag="slab")
                    nc.scalar.dma_start(out=stile[:, 0:(c1 - c0) * F],
                                        in_=slab_d[:, c0 * F:c1 * F])
                    slabs.append(stile)
                if pi < len(oh_b) - 1:
                    c0, c1 = oh_b[pi], oh_b[pi + 1]
                    otile = ohslabpool.tile([128, OHW * 128], bf, tag="ohslab")
                    nc.sync.dma_start(out=otile[:, 0:(c1 - c0) * 128],
                                      in_=ohs_d[:, c0 * 128:c1 * 128])
                    ohslabs.append(otile)
                if pi < len(xrt_b) - 1:
                    c0, c1 = xrt_b[pi], xrt_b[pi + 1]
                    x1 = xrtpool.tile([F1, XRW * 128], bf, tag="xrt1")
                    nc.scalar.dma_start(out=x1[:, 0:(c1 - c0) * 128],
                                        in_=xrT1_d[:, c0 * 128:c1 * 128])
                    x2 = xrtpool.tile([F2, XRW * 128], bf, tag="xrt2")
                    nc.scalar.dma_start(out=x2[:, 0:(c1 - c0) * 128],
                                        in_=xrT2_d[:, c0 * 128:c1 * 128])
                    xrt1s.append(x1)
                    xrt2s.append(x2)

            P1_sb = mpool.tile([F1, T * H], bf)
            P2_sb = mpool.tile([F2, T * H], bf)
            for pp in range(4):
                nc.sync.dma_start(out=P1_sb[:, pp * 1568:(pp + 1) * 1568],
                                  in_=P_d[0:F1, pp * 1568:(pp + 1) * 1568])
                nc.sync.dma_start(out=P2_sb[:, pp * 1568:(pp + 1) * 1568],
                                  in_=P_d[F1:F, pp * 1568:(pp + 1) * 1568])

            for ti in range(NT_TILES):
                tt = ti // TILES_PER_T
                j = ti % TILES_PER_T
                kti = klist[ti]
                spans = ti_spans[ti]
                nmm = kti + len(spans)
                xpi = piece_of_tile[ti]
                x1 = xrt1s[xpi]
                x2 = xrt2s[xpi]
                xo = (ti - xrt_b[xpi]) * 128
                if j == 0:
                    sT1q = stpool.tile([128, S], bf, tag="sT1q")
                    sT2q = stpool.tile([128, S], bf, tag="sT2q")
                if nmm == 0:
                    nc.vector.tensor_copy(out=sT1q[:, j * 128:(j + 1) * 128],
                                          in_=x1[:, xo:xo + 128])
                    nc.scalar.copy(out=sT2q[0:F2, j * 128:(j + 1) * 128],
                                   in_=x2[:, xo:xo + 128])
                else:
                    psum_s = pspool.tile([128, 128], dt, space="PSUM")
                    psum_s2 = ps2pool.tile([F2, 128], dt, space="PSUM")
                    # ops: (lhsT source, oh stream slot) per chunk, F1 pass then F2
                    ops = []
                    for k in range(kti):
                        cidx = col_base[ti] + k
                        spi_ = piece_of_chunk[cidx]
                        ysl = slabs[spi_]
                        off = (cidx - slab_b[spi_]) * F
                        q = oh_of[(0, cidx)]
                        ops.append((ysl, off, q))
                    for d, spcol in spans:
                        ops.append((ydup[d], 0, oh_of[(1, spcol)]))
                    for i, (ysl, off, q) in enumerate(ops):
                        opi = piece_of_oh[q]
                        ohsl = ohslabs[opi]
                        oho = (q - oh_b[opi]) * 128
                        nc.tensor.matmul(out=psum_s[:], lhsT=ysl[:, off:off + F1],
                                         rhs=ohsl[:, oho:oho + 128],
                                         start=i == 0, stop=i == len(ops) - 1)
                    for i, (ysl, off, q) in enumerate(ops):
                        opi = piece_of_oh[q]
                        ohsl = ohslabs[opi]
                        oho = (q - oh_b[opi]) * 128
                        nc.tensor.matmul(out=psum_s2[:], lhsT=ysl[:, off + F1:off + F],
                                         rhs=ohsl[:, oho:oho + 128],
                                         start=i == 0, stop=i == len(ops) - 1)
                    # merge self term during PSUM -> SBUF copy
                    nc.vector.tensor_tensor(
                        out=sT1q[:, j * 128:(j + 1) * 128], in0=psum_s[:],
                        in1=x1[:, xo:xo + 128], op=mybir.AluOpType.add)
                    nc.vector.tensor_tensor(
                        out=sT2q[0:F2, j * 128:(j + 1) * 128], in0=psum_s2[:],
                        in1=x2[:, xo:xo + 128], op=mybir.AluOpType.add)
                if j == TILES_PER_T - 1:
                    # ---- stage 2 batched: z^T = relu(P_t^T s^T + b)
                    pz_a = pzpool.tile([128, 512], dt, space="PSUM", tag="pz")
                    pz_b = pzpool.tile([128, S - 512], dt, space="PSUM", tag="pz")
                    pc0 = tt * H
                    nc.tensor.matmul(out=pz_a[:], lhsT=P1_sb[:, pc0:pc0 + H],
                                     rhs=sT1q[:, 0:512], start=True, stop=False)
                    nc.tensor.matmul(out=pz_b[:], lhsT=P1_sb[:, pc0:pc0 + H],
                                     rhs=sT1q[:, 512:S], start=True, stop=False)
                    nc.tensor.matmul(out=pz_a[:], lhsT=P2_sb[:, pc0:pc0 + H],
                                     rhs=sT2q[0:F2, 0:512], start=False, stop=True)
                    nc.tensor.matmul(out=pz_b[:], lhsT=P2_sb[:, pc0:pc0 + H],
                                     rhs=sT2q[0:F2, 512:S], start=False, stop=True)
                    zTq = ztpool.tile([128, S], bf, tag="zTq")
                    nc.scalar.activation(out=zTq[:, 0:512], in_=pz_a[:],
                                         func=mybir.ActivationFunctionType.Relu,
                                         bias=projb_sb[:, 0:1])
                    nc.scalar.activation(out=zTq[:, 512:S], in_=pz_b[:],
                                         func=mybir.ActivationFunctionType.Relu,
                                         bias=projb_sb[:, 0:1])
                    # ---- stage 3 batched: lg^T for the whole group
                    if tt % GBATCH == 0:
                        lgb = lgpool.tile([C, GBATCH * S], dt, tag="lgb")
                    lo = (tt % GBATCH) * S
                    psum_lg = pzpool.tile([C, 512], dt, space="PSUM", tag="pz")
                    nc.tensor.matmul(out=psum_lg[:], lhsT=clsw_sb[:],
                                     rhs=zTq[:, 0:512], start=True, stop=True)
                    nc.scalar.copy(out=lgb[:, lo:lo + 512], in_=psum_lg[:])
                    psum_lg2 = pzpool.tile([C, 512], dt, space="PSUM", tag="pz")
                    nc.tensor.matmul(out=psum_lg2[:, 0:S - 512], lhsT=clsw_sb[:],
                                     rhs=zTq[:, 512:S], start=True, stop=True)
                    nc.scalar.copy(out=lgb[:, lo + 512:lo + S],
                                   in_=psum_lg2[:, 0:S - 512])
                    if tt % GBATCH == GBATCH - 1 or tt == T - 1:
                        b0 = (tt - tt % GBATCH) * S
                        nc.sync.dma_start(
                            out=lgT_d[:, b0:b0 + (tt % GBATCH) * S + S],
                            in_=lgb[:, 0:(tt % GBATCH) * S + S])
    nc.compile()
    return nc


def kernel(**inputs):
    from concourse.bass_utils import run_bass_kernel_spmd

    np_inputs = {k: np.asarray(v) for k, v in inputs.items()}
    per_core, orig_of, K = _host_prep(**np_inputs)

    if K not in _cache:
        _cache[K] = _build(K)
    nc = _cache[K]

    res = run_bass_kernel_spmd(nc, per_core, list(range(NCORES)))

    cls_b = np_inputs["cls_b"].astype(np.float32)
    logits = np.zeros((N, C), np.float32)
    for c in range(NCORES):
        ids = orig_of[c]
        valid = ids >= 0
        lgT = res.results[c]["lgT"]                    # [C, NPAD]
        logits[ids[valid]] = lgT.T[valid]
    logits += cls_b
    return logits
